# revision 22
# baseline (speedup 1.0000x reference)
"""Multi-head attention (B=2, S=2048, H=1024, 16 heads) on 8 NeuronCores.

Tensor-parallel sharding: 2 heads per core.  Each core computes QKV for its
heads, full attention over the sequence for its heads, and a partial output
projection (its 128 rows of w_dense).  The host sums the 8 partial outputs
(the all-reduce) and adds the output-side bias terms.

Layout notes (per core):
  hsT  [hid, seq]      hidden states transposed (PE transpose), streamed in
                       512-seq windows.
  QT,KT [128, 4096]    q/k transposed: partition = 2 heads x 64 dim,
                       free = global seq (b*2048 + s).
  Vn   [128, 32, 2, 66] v natural: partition = seq within 128-chunk,
                       [chunk, head, dim]; col 64 of the last axis is 1.0 so
                       the P@V matmul also produces the softmax denominator.
  PT   [128, 16, 1024] exp(scores) for one (b, head, q-window):
                       partition = k within chunk, [k-chunk, q].
  ctxT [128, 4096]     unnormalized context transposed (dim on partitions).
  The softmax 1/sum is applied in the dense epilogue (per-partition scalar).
"""

import os
import sys
import types

sys.path.insert(0, "/opt/trn_rl_repo")

import numpy as np


def _install_ntff_shim():
    """The trimmed container image lacks ``antenv.axon_hooks``, which
    ``run_bass_kernel_spmd(trace=True)`` needs to capture NTFF profiles
    under axon.  Recreate it from the boot helper + the injected .so."""
    if "antenv.axon_hooks" in sys.modules:
        return
    try:
        from trn_agent_boot.trn_boot import _ntff_profile_via_ctypes
        so = "/opt/axon/libaxon_pjrt.so"
        if not os.path.exists(so):
            return
        hook = _ntff_profile_via_ctypes(so)
        mod = types.ModuleType("antenv.axon_hooks")
        mod.get_axon_ntff_profile_hook = lambda: hook
        mod.set_axon_ntff_profile_hook = lambda h: None
        sys.modules["antenv.axon_hooks"] = mod
    except Exception:
        pass


_install_ntff_shim()

import concourse.bass as bass
import concourse.mybir as mybir
import concourse.tile as tile
from concourse import bacc
from concourse.bass_utils import run_bass_kernel_spmd
from concourse.masks import make_identity

F32 = mybir.dt.float32
F32R = mybir.dt.float32r
EXP = mybir.ActivationFunctionType.Exp

B, S, HID = 2, 2048, 1024
HEADS, D = 16, 64
SEQ = B * S                      # 4096 flattened rows
NCORES = 8
HPC = HEADS // NCORES            # heads per core = 2
CW = HPC * D                     # per-core width = 128
NHB = HID // 128                 # hidden 128-chunks = 8
WSEQ = 512                       # seq window for transpose+QKV
NWIN = SEQ // WSEQ               # 8
QW = 1024                        # q window in attention
NKT = S // 128                   # k chunks per batch = 16
NCH = SEQ // 128                 # global 128-row chunks = 32


def build_nc():
    nc = bacc.Bacc("TRN2", target_bir_lowering=False, debug=False,
                   num_devices=NCORES)

    hs = nc.dram_tensor("hs", [SEQ, HID], F32, kind="ExternalInput")
    wq = nc.dram_tensor("wq", [HID, CW], F32, kind="ExternalInput")
    wk = nc.dram_tensor("wk", [HID, CW], F32, kind="ExternalInput")
    wv = nc.dram_tensor("wv", [HID, CW], F32, kind="ExternalInput")
    bq = nc.dram_tensor("bq", [CW, 1], F32, kind="ExternalInput")
    bk = nc.dram_tensor("bk", [CW, 1], F32, kind="ExternalInput")
    wd = nc.dram_tensor("wd", [CW, HID], F32, kind="ExternalInput")
    out = nc.dram_tensor("out", [SEQ, HID], F32, kind="ExternalOutput")

    BF16 = mybir.dt.bfloat16

    with tile.TileContext(nc) as tc:
        with (
            tc.tile_pool(name="persist", bufs=1) as pp,
            tc.tile_pool(name="pt", bufs=1) as ptp,
        ):
            ident = pp.tile([128, 128], F32)
            make_identity(nc, ident[:])

            wq_sb = pp.tile([128, NHB, CW], F32R)
            wk_sb = pp.tile([128, NHB, CW], F32R)
            wv_sb = pp.tile([128, NHB, CW], F32R)
            for wsb, wdr in ((wq_sb, wq), (wk_sb, wk), (wv_sb, wv)):
                nc.sync.dma_start(
                    wsb[:],
                    wdr.ap().bitcast(F32R).rearrange("(c p) m -> p c m", p=128))
            bq_sb = pp.tile([CW, 1], F32)
            bk_sb = pp.tile([CW, 1], F32)
            nc.sync.dma_start(bq_sb[:], bq[:])
            nc.sync.dma_start(bk_sb[:], bk[:])
            wd_sb = pp.tile([CW, HID], F32R)
            nc.sync.dma_start(wd_sb[:], wd.ap().bitcast(F32R))

            # Compact per-head layouts: head h occupies partition range
            # h*64..h*64+64 of QT/KT/ctxT.  The scores and dense matmuls
            # run as concurrent 64-row tile pairs (positions (0,0)/(64,0));
            # P@V runs as batched 128-contraction matmuls in bf16.
            QT = pp.tile([CW, SEQ], F32R)
            KT = pp.tile([CW, SEQ], F32R)
            Vn = pp.tile([128, NCH, HPC, 66], BF16)
            ctxT = pp.tile([CW, SEQ], F32R)
            denr = pp.tile([128, NCH, HPC], F32)   # 1/rowsum, [q%128, st, h]
            den2 = pp.tile([D, HPC, QW], F32)      # row 0 = rowsums, rest 0
            PT0 = ptp.tile([128, NKT, QW], BF16)
            PT1 = ptp.tile([128, NKT, QW], BF16)

            nc.vector.memset(den2[:], 0.0)

            # ones column used by the P@V matmul to emit row sums
            ones_st = pp.tile([128, NCH * HPC], F32)
            nc.vector.memset(ones_st[:], 1.0)
            nc.vector.tensor_copy(
                Vn[:, :, :, 64:65],
                ones_st[:].rearrange("p (c h) -> p c h", c=NCH)
                .rearrange("p c h -> p c h ()"))

            # ---------------- phase 1: transpose hs + QKV projections ----
            with (
                tc.tile_pool(name="hsload", bufs=3) as hlp,
                tc.tile_pool(name="hstw", bufs=1) as hwp,
                tc.tile_pool(name="vtw", bufs=2) as vwp,
                tc.tile_pool(name="ps_tr", bufs=2,
                             space=bass.MemorySpace.PSUM) as ptr,
                tc.tile_pool(name="ps_qkv", bufs=2,
                             space=bass.MemorySpace.PSUM) as pqk,
            ):
                hsT = hwp.tile([128, NHB, WSEQ], F32R)
                for w in range(NWIN):
                    r0 = w * WSEQ
                    for sb in range(WSEQ // 128):
                        hsn = hlp.tile([128, HID], F32)
                        nc.sync.dma_start(hsn[:], hs[r0 + sb * 128:
                                                     r0 + (sb + 1) * 128, :])
                        trp = ptr.tile([128, HID], F32, tag="tr")
                        for hb in range(NHB):
                            nc.tensor.transpose(
                                trp[:, hb * 128:(hb + 1) * 128],
                                hsn[:, hb * 128:(hb + 1) * 128],
                                ident[:])
                        if sb % 2 == 0:
                            nc.scalar.copy(
                                hsT[:, :, sb * 128:(sb + 1) * 128],
                                trp[:].rearrange("p (h s) -> p h s", h=NHB))
                        else:
                            nc.vector.tensor_copy(
                                hsT[:, :, sb * 128:(sb + 1) * 128],
                                trp[:].rearrange("p (h s) -> p h s", h=NHB))
                    for tgt in range(3):
                        ps = pqk.tile([128, WSEQ], F32, tag="qkv")
                        wsb = (wq_sb, wk_sb, wv_sb)[tgt]
                        for hb in range(NHB):
                            nc.tensor.matmul(
                                ps[:], wsb[:, hb, :], hsT[:, hb, :],
                                start=(hb == 0), stop=(hb == NHB - 1))
                        if tgt == 0:
                            nc.vector.tensor_scalar_add(
                                QT[:, r0:r0 + WSEQ], ps[:], bq_sb[:, 0:1])
                        elif tgt == 1:
                            nc.vector.tensor_scalar_add(
                                KT[:, r0:r0 + WSEQ], ps[:], bk_sb[:, 0:1])
                        else:
                            vtw = vwp.tile([128, WSEQ], F32)
                            nc.vector.tensor_copy(vtw[:], ps[:])
                            vps = ptr.tile([128, WSEQ], F32, tag="vtr")
                            for sb2 in range(WSEQ // 128):
                                nc.tensor.transpose(
                                    vps[:, sb2 * 128:(sb2 + 1) * 128],
                                    vtw[:, sb2 * 128:(sb2 + 1) * 128],
                                    ident[:])
                            ch0 = r0 // 128
                            nc.vector.tensor_copy(
                                Vn[:, ch0:ch0 + 4, :, 0:64],
                                vps[:].rearrange("p (c h d) -> p c h d",
                                                 c=4, h=HPC))

            # ---------------- phase 2: attention + output projection -----
            # Per (b, qw) iteration: one 64-row-mode batch (scores as
            # concurrent head pairs + previous iteration's dense pairs and
            # denominator extraction), then one 128-mode batch (P@V).
            with (
                tc.tile_pool(name="ps_st", bufs=1,
                             space=bass.MemorySpace.PSUM) as pst,
                tc.tile_pool(name="ps_pv", bufs=1,
                             space=bass.MemorySpace.PSUM) as ppv,
                tc.tile_pool(name="outst", bufs=3) as osp,
            ):
                def dense_and_den(qbase):
                    st0 = qbase // 128
                    # denominators: extract row 0 of den2 transposed via a
                    # basis column, one [64,128]x[64,1] matmul per 128-q
                    for hh in range(HPC):
                        dtag = "st0" if hh == 0 else "st1"
                        dnp = pst.tile([128, QW // 128], F32, tag=dtag)
                        for qt in range(QW // 128):
                            nc.tensor.matmul(
                                dnp[:, qt:qt + 1],
                                den2[:, hh, qt * 128:(qt + 1) * 128],
                                ident[0:D, 0:1],
                                start=True, stop=True,
                                tile_position=(0, 0))
                        nc.vector.reciprocal(
                            denr[:, st0:st0 + QW // 128, hh:hh + 1], dnp[:])
                    for stl in range(QW // 128):
                        st = qbase // 128 + stl
                        ssl = slice(st * 128, (st + 1) * 128)
                        for nt in range(HID // 512):
                            nsl = slice(nt * 512, (nt + 1) * 512)
                            psa = pst.tile([128, 512], F32, tag="st0")
                            psb = pst.tile([128, 512], F32, tag="st1")
                            nc.tensor.matmul(
                                psa[:], ctxT[0:D, ssl], wd_sb[0:D, nsl],
                                start=True, stop=True, tile_position=(0, 0))
                            nc.tensor.matmul(
                                psb[:], ctxT[D:2 * D, ssl],
                                wd_sb[D:2 * D, nsl],
                                start=True, stop=True, tile_position=(64, 0))
                            ob = osp.tile([128, 512], F32)
                            nc.vector.tensor_scalar_mul(
                                ob[:], psa[:], denr[:, st, 0:1])
                            ob2 = osp.tile([128, 512], F32, tag="ob2")
                            nc.vector.scalar_tensor_tensor(
                                ob2[:], psb[:], denr[:, st, 1:2], ob[:],
                                op0=mybir.AluOpType.mult,
                                op1=mybir.AluOpType.add)
                            nc.sync.dma_start(out[ssl, nsl], ob2[:])

                prev = None
                for b in range(B):
                    for qw in range(S // QW):
                        qbase = b * S + qw * QW
                        qsl = slice(qbase, qbase + QW)
                        # --- 64-row-mode batch: scores head-pairs ---
                        for kt in range(NKT):
                            ksl = slice(b * S + kt * 128,
                                        b * S + (kt + 1) * 128)
                            stp0 = pst.tile([128, QW], F32, tag="st0")
                            stp1 = pst.tile([128, QW], F32, tag="st1")
                            for qh in range(QW // 512):
                                sl = slice(qh * 512, (qh + 1) * 512)
                                qgl = slice(qbase + qh * 512,
                                            qbase + (qh + 1) * 512)
                                nc.tensor.matmul(
                                    stp0[:, sl], KT[0:D, ksl], QT[0:D, qgl],
                                    start=True, stop=True,
                                    tile_position=(0, 0))
                                nc.tensor.matmul(
                                    stp1[:, sl], KT[D:2 * D, ksl],
                                    QT[D:2 * D, qgl],
                                    start=True, stop=True,
                                    tile_position=(64, 0))
                            nc.scalar.activation(
                                PT0[:, kt, :], stp0[:], EXP, scale=0.125)
                            nc.scalar.activation(
                                PT1[:, kt, :], stp1[:], EXP, scale=0.125)
                            if kt == 0 and prev is not None:
                                dense_and_den(prev)
                        # --- 128-mode batch: P@V per head ---
                        for hh, PTh in ((0, PT0), (1, PT1)):
                            pvp = ppv.tile([D + 1, QW], F32,
                                           tag="pv0" if hh == 0 else "pv1")
                            for kt in range(NKT):
                                ch = b * NKT + kt
                                for qh in range(QW // 512):
                                    sl = slice(qh * 512, (qh + 1) * 512)
                                    nc.tensor.matmul(
                                        pvp[:, sl], Vn[:, ch, hh, 0:65],
                                        PTh[:, kt, sl],
                                        start=(kt == 0),
                                        stop=(kt == NKT - 1))
                            nc.vector.tensor_copy(
                                ctxT[hh * D:(hh + 1) * D, qsl], pvp[0:D, :])
                            nc.vector.tensor_copy(
                                den2[0:1, hh, :], pvp[D:D + 1, :])
                        prev = qbase
                dense_and_den(prev)

    nc.compile()
    return nc


_NC_CACHE = None


def get_nc():
    global _NC_CACHE
    if _NC_CACHE is None:
        _NC_CACHE = build_nc()
    return _NC_CACHE


def make_in_maps(hidden_states, w_qkv, b_qkv, w_dense):
    hs = np.ascontiguousarray(
        np.asarray(hidden_states, dtype=np.float32).reshape(SEQ, HID))
    w_qkv = np.asarray(w_qkv, dtype=np.float32)
    b_qkv = np.asarray(b_qkv, dtype=np.float32)
    w_dense = np.asarray(w_dense, dtype=np.float32)
    # Reference layout: qkv.reshape(B, S, HEADS, 3*D) split on the last
    # axis, i.e. w_qkv columns are per-head [q_h | k_h | v_h] blocks of D.
    wq_cols = np.concatenate(
        [np.arange(h * 3 * D, h * 3 * D + D) for h in range(HEADS)])
    wk_cols = wq_cols + D
    wv_cols = wq_cols + 2 * D
    in_maps = []
    for c in range(NCORES):
        c0 = c * CW
        sel = slice(c0, c0 + CW)
        in_maps.append({
            "hs": hs,
            "wq": np.ascontiguousarray(w_qkv[:, wq_cols[sel]]),
            "wk": np.ascontiguousarray(w_qkv[:, wk_cols[sel]]),
            "wv": np.ascontiguousarray(w_qkv[:, wv_cols[sel]]),
            "bq": np.ascontiguousarray(b_qkv[wq_cols[sel]].reshape(CW, 1)),
            "bk": np.ascontiguousarray(b_qkv[wk_cols[sel]].reshape(CW, 1)),
            "wd": np.ascontiguousarray(w_dense[sel, :]),
        })
    return in_maps


def run(hidden_states, w_qkv, b_qkv, w_dense, b_dense, trace=False):
    nc = get_nc()
    in_maps = make_in_maps(hidden_states, w_qkv, b_qkv, w_dense)
    res = run_bass_kernel_spmd(nc, in_maps, core_ids=list(range(NCORES)),
                               trace=trace)
    acc = res.results[0]["out"].astype(np.float32)
    for c in range(1, NCORES):
        acc = acc + res.results[c]["out"]
    # bias terms that commute to the end: v-bias through dense, dense bias
    b_qkv = np.asarray(b_qkv, dtype=np.float32)
    b_v = np.concatenate(
        [b_qkv[h * 3 * D + 2 * D:h * 3 * D + 3 * D] for h in range(HEADS)])
    acc = acc + (b_v @ np.asarray(w_dense, dtype=np.float32)
                 + np.asarray(b_dense, dtype=np.float32))
    return acc.reshape(B, S, HID).astype(np.float32), res


def kernel(hidden_states, w_qkv, b_qkv, w_dense, b_dense):
    out, _ = run(hidden_states, w_qkv, b_qkv, w_dense, b_dense,
                 trace=bool(os.environ.get("BASS_TRACE")))
    return out


# revision 26
# speedup vs baseline: 1.2751x; 1.2751x over previous
"""Multi-head attention (B=2, S=2048, H=1024, 16 heads) on 8 NeuronCores.

Tensor-parallel sharding: 2 heads per core.  Each core computes QKV for its
heads, full attention over the sequence for its heads, and a partial output
projection (its 128 rows of w_dense).  The host sums the 8 partial outputs
(the all-reduce) and adds the output-side bias terms.

Layout notes (per core):
  hsT  [hid, seq]      hidden states transposed (PE transpose), streamed in
                       512-seq windows.
  QT,KT [128, 4096]    q/k transposed: partition = 2 heads x 64 dim,
                       free = global seq (b*2048 + s).
  Vn   [128, 32, 2, 66] v natural: partition = seq within 128-chunk,
                       [chunk, head, dim]; col 64 of the last axis is 1.0 so
                       the P@V matmul also produces the softmax denominator.
  PT   [128, 16, 1024] exp(scores) for one (b, head, q-window):
                       partition = k within chunk, [k-chunk, q].
  ctxT [128, 4096]     unnormalized context transposed (dim on partitions).
  The softmax 1/sum is applied in the dense epilogue (per-partition scalar).
"""

import os
import sys
import types

sys.path.insert(0, "/opt/trn_rl_repo")

import numpy as np


def _install_ntff_shim():
    """The trimmed container image lacks ``antenv.axon_hooks``, which
    ``run_bass_kernel_spmd(trace=True)`` needs to capture NTFF profiles
    under axon.  Recreate it from the boot helper + the injected .so."""
    if "antenv.axon_hooks" in sys.modules:
        return
    try:
        from trn_agent_boot.trn_boot import _ntff_profile_via_ctypes
        so = "/opt/axon/libaxon_pjrt.so"
        if not os.path.exists(so):
            return
        hook = _ntff_profile_via_ctypes(so)
        mod = types.ModuleType("antenv.axon_hooks")
        mod.get_axon_ntff_profile_hook = lambda: hook
        mod.set_axon_ntff_profile_hook = lambda h: None
        sys.modules["antenv.axon_hooks"] = mod
    except Exception:
        pass


_install_ntff_shim()

import concourse.bass as bass
import concourse.mybir as mybir
import concourse.tile as tile
from concourse import bacc
from concourse.bass_utils import run_bass_kernel_spmd
from concourse.masks import make_identity

F32 = mybir.dt.float32
F32R = mybir.dt.float32r
EXP = mybir.ActivationFunctionType.Exp

B, S, HID = 2, 2048, 1024
HEADS, D = 16, 64
SEQ = B * S                      # 4096 flattened rows
NCORES = 8
HPC = HEADS // NCORES            # heads per core = 2
CW = HPC * D                     # per-core width = 128
NHB = HID // 128                 # hidden 128-chunks = 8
WSEQ = 512                       # seq window for transpose+QKV
NWIN = SEQ // WSEQ               # 8
QW = 1024                        # q window in attention
NKT = S // 128                   # k chunks per batch = 16
NCH = SEQ // 128                 # global 128-row chunks = 32


def build_nc():
    nc = bacc.Bacc("TRN2", target_bir_lowering=False, debug=False,
                   num_devices=NCORES)

    hs = nc.dram_tensor("hs", [SEQ, HID], F32, kind="ExternalInput")
    wq = nc.dram_tensor("wq", [HID, CW], F32, kind="ExternalInput")
    wk = nc.dram_tensor("wk", [HID, CW], F32, kind="ExternalInput")
    wv = nc.dram_tensor("wv", [HID, CW], F32, kind="ExternalInput")
    bq = nc.dram_tensor("bq", [CW, 1], F32, kind="ExternalInput")
    bk = nc.dram_tensor("bk", [CW, 1], F32, kind="ExternalInput")
    wd = nc.dram_tensor("wd", [CW, HID], F32, kind="ExternalInput")
    out = nc.dram_tensor("out", [SEQ, HID], F32, kind="ExternalOutput")

    RING = 4

    with tile.TileContext(nc) as tc:
        with (
            tc.tile_pool(name="persist", bufs=1) as pp,
            tc.tile_pool(name="pt", bufs=1) as ptp,
        ):
            ident = pp.tile([128, 128], F32)
            make_identity(nc, ident[:])

            wq_sb = pp.tile([128, NHB, CW], F32R)
            wk_sb = pp.tile([128, NHB, CW], F32R)
            wv_sb = pp.tile([128, NHB, CW], F32R)
            for wsb, wdr in ((wq_sb, wq), (wk_sb, wk), (wv_sb, wv)):
                nc.sync.dma_start(
                    wsb[:],
                    wdr.ap().bitcast(F32R).rearrange("(c p) m -> p c m", p=128))
            bq_sb = pp.tile([CW, 1], F32)
            bk_sb = pp.tile([CW, 1], F32)
            nc.sync.dma_start(bq_sb[:], bq[:])
            nc.sync.dma_start(bk_sb[:], bk[:])
            wd_sb = pp.tile([CW, HID], F32R)
            nc.sync.dma_start(wd_sb[:], wd.ap().bitcast(F32R))

            # Everything on the PE stays in plain 128x128 mode.  Per-head
            # operands are zero-padded to a full 128-partition contraction:
            #   QTz/KTz [:, h, :]  rows 0-63 = head h, rows 64-127 = 0
            #   ctxTz   [:, 0, :]  rows 0-63 = head 0 ctx, upper rows 0
            #   ctxTz   [:, 1, :]  rows 64-127 = head 1 ctx, lower rows 0
            # so the dense matmul can take full-width w_dense slices.
            QTz = pp.tile([128, HPC, SEQ], F32R)
            KTz = pp.tile([128, HPC, SEQ], F32R)
            Vn = pp.tile([128, NCH, HPC, 66], F32R)
            ctxTz = pp.tile([128, HPC, SEQ], F32R)
            denr = pp.tile([128, NCH, HPC], F32)   # 1/rowsum, [q%128, st, h]
            den2 = pp.tile([128, HPC, QW], F32)    # row 0 = rowsums, rest 0
            PT = ptp.tile([128, RING, QW], F32R)

            nc.vector.memset(den2[:], 0.0)

            # ones column used by the P@V matmul to emit row sums
            ones_st = pp.tile([128, NCH * HPC], F32)
            nc.vector.memset(ones_st[:], 1.0)
            nc.vector.tensor_copy(
                Vn[:, :, :, 64:65],
                ones_st[:].rearrange("p (c h) -> p c h", c=NCH)
                .rearrange("p c h -> p c h ()"))

            # ---------------- phase 1: transpose hs + QKV projections ----
            with (
                tc.tile_pool(name="hsload", bufs=3) as hlp,
                tc.tile_pool(name="hstw", bufs=1) as hwp,
                tc.tile_pool(name="vtw", bufs=2) as vwp,
                tc.tile_pool(name="zs", bufs=1) as zsp,
                tc.tile_pool(name="ps_tr", bufs=2,
                             space=bass.MemorySpace.PSUM) as ptr,
                tc.tile_pool(name="ps_qkv", bufs=2,
                             space=bass.MemorySpace.PSUM) as pqk,
            ):
                # zero-fill the padded halves (f32r memset is rejected by
                # the ISA checker, so bounce through an f32 staging tile)
                zs = zsp.tile([D, SEQ], F32)
                nc.vector.memset(zs[:], 0.0)
                for h in range(HPC):
                    nc.vector.tensor_copy(QTz[D:128, h, :], zs[:])
                    nc.vector.tensor_copy(KTz[D:128, h, :], zs[:])
                nc.vector.tensor_copy(ctxTz[D:128, 0, :], zs[:])
                nc.vector.tensor_copy(ctxTz[0:D, 1, :], zs[:])

                hsT = hwp.tile([128, NHB, WSEQ], F32R)
                for w in range(NWIN):
                    r0 = w * WSEQ
                    for sb in range(WSEQ // 128):
                        hsn = hlp.tile([128, HID], F32)
                        nc.sync.dma_start(hsn[:], hs[r0 + sb * 128:
                                                     r0 + (sb + 1) * 128, :])
                        trp = ptr.tile([128, HID], F32, tag="tr")
                        for hb in range(NHB):
                            nc.tensor.transpose(
                                trp[:, hb * 128:(hb + 1) * 128],
                                hsn[:, hb * 128:(hb + 1) * 128],
                                ident[:])
                        eng = nc.scalar if sb % 2 == 0 else nc.vector
                        if sb % 2 == 0:
                            nc.scalar.copy(
                                hsT[:, :, sb * 128:(sb + 1) * 128],
                                trp[:].rearrange("p (h s) -> p h s", h=NHB))
                        else:
                            nc.vector.tensor_copy(
                                hsT[:, :, sb * 128:(sb + 1) * 128],
                                trp[:].rearrange("p (h s) -> p h s", h=NHB))
                    for tgt in range(3):
                        ps = pqk.tile([128, WSEQ], F32, tag="qkv")
                        wsb = (wq_sb, wk_sb, wv_sb)[tgt]
                        for hb in range(NHB):
                            nc.tensor.matmul(
                                ps[:], wsb[:, hb, :], hsT[:, hb, :],
                                start=(hb == 0), stop=(hb == NHB - 1))
                        if tgt < 2:
                            dst, bias = ((QTz, bq_sb), (KTz, bk_sb))[tgt]
                            for h in range(HPC):
                                nc.vector.tensor_scalar_add(
                                    dst[0:D, h, r0:r0 + WSEQ],
                                    ps[h * D:(h + 1) * D, :],
                                    bias[h * D:(h + 1) * D, 0:1])
                        else:
                            vtw = vwp.tile([128, WSEQ], F32)
                            nc.vector.tensor_copy(vtw[:], ps[:])
                            vps = ptr.tile([128, WSEQ], F32, tag="vtr")
                            for sb2 in range(WSEQ // 128):
                                nc.tensor.transpose(
                                    vps[:, sb2 * 128:(sb2 + 1) * 128],
                                    vtw[:, sb2 * 128:(sb2 + 1) * 128],
                                    ident[:])
                            ch0 = r0 // 128
                            nc.vector.tensor_copy(
                                Vn[:, ch0:ch0 + 4, :, 0:64],
                                vps[:].rearrange("p (c h d) -> p c h d",
                                                 c=4, h=HPC))

            # ---------------- phase 2: attention + output projection -----
            with (
                tc.tile_pool(name="ps_st", bufs=3,
                             space=bass.MemorySpace.PSUM) as pst,
                tc.tile_pool(name="ps_pv", bufs=1,
                             space=bass.MemorySpace.PSUM) as ppv,
                tc.tile_pool(name="outst", bufs=3) as osp,
            ):
                for b in range(B):
                    for qw in range(S // QW):
                        qbase = b * S + qw * QW
                        qsl = slice(qbase, qbase + QW)
                        st0 = qbase // 128
                        for hh in range(HPC):
                            pvp = ppv.tile([D + 1, QW], F32, tag="pv")
                            for kt in range(NKT):
                                ch = b * NKT + kt
                                ksl = slice(b * S + kt * 128,
                                            b * S + (kt + 1) * 128)
                                rg = kt % RING
                                stp = pst.tile([128, QW], F32, tag="st")
                                for qh in range(QW // 512):
                                    sl = slice(qh * 512, (qh + 1) * 512)
                                    nc.tensor.matmul(
                                        stp[:, sl], KTz[:, hh, ksl],
                                        QTz[:, hh,
                                            qbase + qh * 512:
                                            qbase + (qh + 1) * 512],
                                        start=True, stop=True)
                                nc.scalar.activation(
                                    PT[:, rg, :], stp[:], EXP, scale=0.125)
                                for qh in range(QW // 512):
                                    sl = slice(qh * 512, (qh + 1) * 512)
                                    nc.tensor.matmul(
                                        pvp[:, sl], Vn[:, ch, hh, 0:65],
                                        PT[:, rg, sl],
                                        start=(kt == 0),
                                        stop=(kt == NKT - 1))
                            # ctx into its head's partition range; keep the
                            # denominator row for the normalization pass
                            nc.vector.tensor_copy(
                                ctxTz[hh * D:(hh + 1) * D, hh, qsl],
                                pvp[0:D, :])
                            nc.vector.tensor_copy(
                                den2[0:1, hh, :], pvp[D:D + 1, :])
                            # transpose the rowsum row via basis column
                            dnp = pst.tile([128, QW // 128], F32, tag="st")
                            for qt in range(QW // 128):
                                nc.tensor.matmul(
                                    dnp[:, qt:qt + 1],
                                    den2[:, hh, qt * 128:(qt + 1) * 128],
                                    ident[:, 0:1],
                                    start=True, stop=True)
                            nc.vector.reciprocal(
                                denr[:, st0:st0 + QW // 128, hh:hh + 1],
                                dnp[:])
                        # dense partial; per-head normalization as
                        # per-partition scalars in the epilogue
                        for stl in range(QW // 128):
                            st = qbase // 128 + stl
                            ssl = slice(st * 128, (st + 1) * 128)
                            for nt in range(HID // 512):
                                nsl = slice(nt * 512, (nt + 1) * 512)
                                psa = pst.tile([128, 512], F32, tag="st")
                                nc.tensor.matmul(
                                    psa[:], ctxTz[:, 0, ssl], wd_sb[:, nsl],
                                    start=True, stop=True)
                                psb = pst.tile([128, 512], F32, tag="st")
                                nc.tensor.matmul(
                                    psb[:], ctxTz[:, 1, ssl], wd_sb[:, nsl],
                                    start=True, stop=True)
                                ob = osp.tile([128, 512], F32)
                                nc.vector.tensor_scalar_mul(
                                    ob[:], psa[:], denr[:, st, 0:1])
                                ob2 = osp.tile([128, 512], F32, tag="ob2")
                                nc.vector.scalar_tensor_tensor(
                                    ob2[:], psb[:], denr[:, st, 1:2], ob[:],
                                    op0=mybir.AluOpType.mult,
                                    op1=mybir.AluOpType.add)
                                nc.sync.dma_start(
                                    out[ssl, nsl], ob2[:])

    nc.compile()
    return nc


_NC_CACHE = None


def get_nc():
    global _NC_CACHE
    if _NC_CACHE is None:
        _NC_CACHE = build_nc()
    return _NC_CACHE


def make_in_maps(hidden_states, w_qkv, b_qkv, w_dense):
    hs = np.ascontiguousarray(
        np.asarray(hidden_states, dtype=np.float32).reshape(SEQ, HID))
    w_qkv = np.asarray(w_qkv, dtype=np.float32)
    b_qkv = np.asarray(b_qkv, dtype=np.float32)
    w_dense = np.asarray(w_dense, dtype=np.float32)
    # Reference layout: qkv.reshape(B, S, HEADS, 3*D) split on the last
    # axis, i.e. w_qkv columns are per-head [q_h | k_h | v_h] blocks of D.
    wq_cols = np.concatenate(
        [np.arange(h * 3 * D, h * 3 * D + D) for h in range(HEADS)])
    wk_cols = wq_cols + D
    wv_cols = wq_cols + 2 * D
    in_maps = []
    for c in range(NCORES):
        c0 = c * CW
        sel = slice(c0, c0 + CW)
        in_maps.append({
            "hs": hs,
            "wq": np.ascontiguousarray(w_qkv[:, wq_cols[sel]]),
            "wk": np.ascontiguousarray(w_qkv[:, wk_cols[sel]]),
            "wv": np.ascontiguousarray(w_qkv[:, wv_cols[sel]]),
            "bq": np.ascontiguousarray(b_qkv[wq_cols[sel]].reshape(CW, 1)),
            "bk": np.ascontiguousarray(b_qkv[wk_cols[sel]].reshape(CW, 1)),
            "wd": np.ascontiguousarray(w_dense[sel, :]),
        })
    return in_maps


def run(hidden_states, w_qkv, b_qkv, w_dense, b_dense, trace=False):
    nc = get_nc()
    in_maps = make_in_maps(hidden_states, w_qkv, b_qkv, w_dense)
    res = run_bass_kernel_spmd(nc, in_maps, core_ids=list(range(NCORES)),
                               trace=trace)
    acc = res.results[0]["out"].astype(np.float32)
    for c in range(1, NCORES):
        acc = acc + res.results[c]["out"]
    # bias terms that commute to the end: v-bias through dense, dense bias
    b_qkv = np.asarray(b_qkv, dtype=np.float32)
    b_v = np.concatenate(
        [b_qkv[h * 3 * D + 2 * D:h * 3 * D + 3 * D] for h in range(HEADS)])
    acc = acc + (b_v @ np.asarray(w_dense, dtype=np.float32)
                 + np.asarray(b_dense, dtype=np.float32))
    return acc.reshape(B, S, HID).astype(np.float32), res


def kernel(hidden_states, w_qkv, b_qkv, w_dense, b_dense):
    out, _ = run(hidden_states, w_qkv, b_qkv, w_dense, b_dense,
                 trace=bool(os.environ.get("BASS_TRACE")))
    return out


# revision 27
# speedup vs baseline: 1.2872x; 1.0095x over previous
"""Multi-head attention (B=2, S=2048, H=1024, 16 heads) on 8 NeuronCores.

Tensor-parallel sharding: 2 heads per core.  Each core computes QKV for its
heads, full attention over the sequence for its heads, and a partial output
projection (its 128 rows of w_dense).  The host sums the 8 partial outputs
(the all-reduce) and adds the output-side bias terms.

Layout notes (per core), all PE matmuls in plain 128x128 mode (mixing
64-row tiled and 128-row matmuls mode-thrashes the PE and halves its
clock, measured):
  hsT  [hid, seq]       hidden states transposed (PE transpose), streamed
                        in 512-seq windows.
  QTz/KTz [128, h, seq] q/k transposed per head, zero-padded to a full
                        128-partition contraction (rows 64-127 = 0).
  Vn  [128, 32, 2, 66]  v natural: partition = seq within 128-chunk,
                        [chunk, head, dim]; col 64 is 1.0 so the P@V
                        matmul also emits the softmax denominators.
  PT  [128, RING, 1024] exp(scores) ring: partition = k within chunk.
  ctxTz [128, h, seq]   unnormalized context transposed; head 0 in rows
                        0-63, head 1 in rows 64-127, other half zero, so
                        the dense matmul takes full w_dense slices.
  Softmax 1/sums are extracted by a basis-column matmul, reciprocals run
  wide on [128, 8], and the normalization lands in the dense epilogue as
  per-partition, per-head scalars.
"""

import os
import sys
import types

sys.path.insert(0, "/opt/trn_rl_repo")

import numpy as np


def _install_ntff_shim():
    """The trimmed container image lacks ``antenv.axon_hooks``, which
    ``run_bass_kernel_spmd(trace=True)`` needs to capture NTFF profiles
    under axon.  Recreate it from the boot helper + the injected .so."""
    if "antenv.axon_hooks" in sys.modules:
        return
    try:
        from trn_agent_boot.trn_boot import _ntff_profile_via_ctypes
        so = "/opt/axon/libaxon_pjrt.so"
        if not os.path.exists(so):
            return
        hook = _ntff_profile_via_ctypes(so)
        mod = types.ModuleType("antenv.axon_hooks")
        mod.get_axon_ntff_profile_hook = lambda: hook
        mod.set_axon_ntff_profile_hook = lambda h: None
        sys.modules["antenv.axon_hooks"] = mod
    except Exception:
        pass


_install_ntff_shim()

import concourse.bass as bass
import concourse.mybir as mybir
import concourse.tile as tile
from concourse import bacc
from concourse.bass_utils import run_bass_kernel_spmd
from concourse.masks import make_identity

F32 = mybir.dt.float32
F32R = mybir.dt.float32r
EXP = mybir.ActivationFunctionType.Exp

B, S, HID = 2, 2048, 1024
HEADS, D = 16, 64
SEQ = B * S                      # 4096 flattened rows
NCORES = 8
HPC = HEADS // NCORES            # heads per core = 2
CW = HPC * D                     # per-core width = 128
NHB = HID // 128                 # hidden 128-chunks = 8
WSEQ = 512                       # seq window for transpose+QKV
NWIN = SEQ // WSEQ               # 8
QW = 1024                        # q window in attention
NKT = S // 128                   # k chunks per batch = 16
NCH = SEQ // 128                 # global 128-row chunks = 32


def build_nc():
    nc = bacc.Bacc("TRN2", target_bir_lowering=False, debug=False,
                   num_devices=NCORES)

    hs = nc.dram_tensor("hs", [SEQ, HID], F32, kind="ExternalInput")
    wq = nc.dram_tensor("wq", [HID, CW], F32, kind="ExternalInput")
    wk = nc.dram_tensor("wk", [HID, CW], F32, kind="ExternalInput")
    wv = nc.dram_tensor("wv", [HID, CW], F32, kind="ExternalInput")
    bq = nc.dram_tensor("bq", [CW, 1], F32, kind="ExternalInput")
    bk = nc.dram_tensor("bk", [CW, 1], F32, kind="ExternalInput")
    wd = nc.dram_tensor("wd", [CW, HID], F32, kind="ExternalInput")
    out = nc.dram_tensor("out", [SEQ, HID], F32, kind="ExternalOutput")

    RING = 4

    with tile.TileContext(nc) as tc:
        with (
            tc.tile_pool(name="persist", bufs=1) as pp,
            tc.tile_pool(name="pt", bufs=1) as ptp,
        ):
            ident = pp.tile([128, 128], F32)
            make_identity(nc, ident[:])

            wq_sb = pp.tile([128, NHB, CW], F32R)
            wk_sb = pp.tile([128, NHB, CW], F32R)
            wv_sb = pp.tile([128, NHB, CW], F32R)
            for wsb, wdr in ((wq_sb, wq), (wk_sb, wk), (wv_sb, wv)):
                nc.sync.dma_start(
                    wsb[:],
                    wdr.ap().bitcast(F32R).rearrange("(c p) m -> p c m", p=128))
            bq_sb = pp.tile([CW, 1], F32)
            bk_sb = pp.tile([CW, 1], F32)
            nc.sync.dma_start(bq_sb[:], bq[:])
            nc.sync.dma_start(bk_sb[:], bk[:])
            wd_sb = pp.tile([CW, HID], F32R)
            nc.sync.dma_start(wd_sb[:], wd.ap().bitcast(F32R))

            # Everything on the PE stays in plain 128x128 mode.  Per-head
            # operands are zero-padded to a full 128-partition contraction:
            #   QTz/KTz [:, h, :]  rows 0-63 = head h, rows 64-127 = 0
            #   ctxTz   [:, 0, :]  rows 0-63 = head 0 ctx, upper rows 0
            #   ctxTz   [:, 1, :]  rows 64-127 = head 1 ctx, lower rows 0
            # so the dense matmul can take full-width w_dense slices.
            QTz = pp.tile([128, HPC, SEQ], F32R)
            KTz = pp.tile([128, HPC, SEQ], F32R)
            Vn = pp.tile([128, NCH, HPC, 66], F32R)
            ctxTz = pp.tile([128, HPC, SEQ], F32R)
            denr = pp.tile([128, NCH, HPC], F32)   # 1/rowsum, [q%128, st, h]
            den2 = pp.tile([128, HPC, QW], F32)    # row 0 = rowsums, rest 0
            PT = ptp.tile([128, RING, QW], F32R)

            nc.vector.memset(den2[:], 0.0)

            # ones column used by the P@V matmul to emit row sums
            ones_st = pp.tile([128, NCH * HPC], F32)
            nc.vector.memset(ones_st[:], 1.0)
            nc.vector.tensor_copy(
                Vn[:, :, :, 64:65],
                ones_st[:].rearrange("p (c h) -> p c h", c=NCH)
                .rearrange("p c h -> p c h ()"))

            # ---------------- phase 1: transpose hs + QKV projections ----
            with (
                tc.tile_pool(name="hsload", bufs=3) as hlp,
                tc.tile_pool(name="hstw", bufs=1) as hwp,
                tc.tile_pool(name="vtw", bufs=2) as vwp,
                tc.tile_pool(name="zs", bufs=1) as zsp,
                tc.tile_pool(name="ps_tr", bufs=2,
                             space=bass.MemorySpace.PSUM) as ptr,
                tc.tile_pool(name="ps_qkv", bufs=2,
                             space=bass.MemorySpace.PSUM) as pqk,
            ):
                # zero-fill the padded halves (f32r memset is rejected by
                # the ISA checker, so bounce through an f32 staging tile)
                zs = zsp.tile([D, SEQ], F32)
                nc.vector.memset(zs[:], 0.0)
                for h in range(HPC):
                    nc.vector.tensor_copy(QTz[D:128, h, :], zs[:])
                    nc.vector.tensor_copy(KTz[D:128, h, :], zs[:])
                nc.vector.tensor_copy(ctxTz[D:128, 0, :], zs[:])
                nc.vector.tensor_copy(ctxTz[0:D, 1, :], zs[:])

                hsT = hwp.tile([128, NHB, WSEQ], F32R)
                for w in range(NWIN):
                    r0 = w * WSEQ
                    for sb in range(WSEQ // 128):
                        hsn = hlp.tile([128, HID], F32)
                        nc.sync.dma_start(hsn[:], hs[r0 + sb * 128:
                                                     r0 + (sb + 1) * 128, :])
                        trp = ptr.tile([128, HID], F32, tag="tr")
                        for hb in range(NHB):
                            nc.tensor.transpose(
                                trp[:, hb * 128:(hb + 1) * 128],
                                hsn[:, hb * 128:(hb + 1) * 128],
                                ident[:])
                        if sb % 2 == 0:
                            nc.scalar.copy(
                                hsT[:, :, sb * 128:(sb + 1) * 128],
                                trp[:].rearrange("p (h s) -> p h s", h=NHB))
                        else:
                            nc.vector.tensor_copy(
                                hsT[:, :, sb * 128:(sb + 1) * 128],
                                trp[:].rearrange("p (h s) -> p h s", h=NHB))
                    for tgt in range(3):
                        ps = pqk.tile([128, WSEQ], F32, tag="qkv")
                        wsb = (wq_sb, wk_sb, wv_sb)[tgt]
                        for hb in range(NHB):
                            nc.tensor.matmul(
                                ps[:], wsb[:, hb, :], hsT[:, hb, :],
                                start=(hb == 0), stop=(hb == NHB - 1))
                        if tgt < 2:
                            dst, bias = ((QTz, bq_sb), (KTz, bk_sb))[tgt]
                            for h in range(HPC):
                                nc.vector.tensor_scalar_add(
                                    dst[0:D, h, r0:r0 + WSEQ],
                                    ps[h * D:(h + 1) * D, :],
                                    bias[h * D:(h + 1) * D, 0:1])
                        else:
                            vtw = vwp.tile([128, WSEQ], F32)
                            nc.vector.tensor_copy(vtw[:], ps[:])
                            vps = ptr.tile([128, WSEQ], F32, tag="vtr")
                            for sb2 in range(WSEQ // 128):
                                nc.tensor.transpose(
                                    vps[:, sb2 * 128:(sb2 + 1) * 128],
                                    vtw[:, sb2 * 128:(sb2 + 1) * 128],
                                    ident[:])
                            ch0 = r0 // 128
                            nc.vector.tensor_copy(
                                Vn[:, ch0:ch0 + 4, :, 0:64],
                                vps[:].rearrange("p (c h d) -> p c h d",
                                                 c=4, h=HPC))

            # ---------------- phase 2: attention + output projection -----
            with (
                tc.tile_pool(name="ps_st", bufs=3,
                             space=bass.MemorySpace.PSUM) as pst,
                tc.tile_pool(name="ps_pv", bufs=1,
                             space=bass.MemorySpace.PSUM) as ppv,
                tc.tile_pool(name="outst", bufs=3) as osp,
            ):
                for b in range(B):
                    for qw in range(S // QW):
                        qbase = b * S + qw * QW
                        qsl = slice(qbase, qbase + QW)
                        st0 = qbase // 128
                        for hh in range(HPC):
                            pvp = ppv.tile([D + 1, QW], F32, tag="pv")
                            for kt in range(NKT):
                                ch = b * NKT + kt
                                ksl = slice(b * S + kt * 128,
                                            b * S + (kt + 1) * 128)
                                rg = kt % RING
                                stp = pst.tile([128, QW], F32, tag="st")
                                for qh in range(QW // 512):
                                    sl = slice(qh * 512, (qh + 1) * 512)
                                    nc.tensor.matmul(
                                        stp[:, sl], KTz[:, hh, ksl],
                                        QTz[:, hh,
                                            qbase + qh * 512:
                                            qbase + (qh + 1) * 512],
                                        start=True, stop=True)
                                nc.scalar.activation(
                                    PT[:, rg, :], stp[:], EXP, scale=0.125)
                                for qh in range(QW // 512):
                                    sl = slice(qh * 512, (qh + 1) * 512)
                                    nc.tensor.matmul(
                                        pvp[:, sl], Vn[:, ch, hh, 0:65],
                                        PT[:, rg, sl],
                                        start=(kt == 0),
                                        stop=(kt == NKT - 1))
                            # ctx into its head's partition range; keep the
                            # denominator row for the normalization pass
                            nc.vector.tensor_copy(
                                ctxTz[hh * D:(hh + 1) * D, hh, qsl],
                                pvp[0:D, :])
                            nc.vector.tensor_copy(
                                den2[0:1, hh, :], pvp[D:D + 1, :])
                            # transpose the rowsum row via basis column
                            dnp = pst.tile([128, QW // 128], F32, tag="st")
                            for qt in range(QW // 128):
                                nc.tensor.matmul(
                                    dnp[:, qt:qt + 1],
                                    den2[:, hh, qt * 128:(qt + 1) * 128],
                                    ident[:, 0:1],
                                    start=True, stop=True)
                            nc.vector.reciprocal(
                                denr[:, st0:st0 + QW // 128, hh:hh + 1],
                                dnp[:])
                        # dense partial; per-head normalization as
                        # per-partition scalars in the epilogue
                        for stl in range(QW // 128):
                            st = qbase // 128 + stl
                            ssl = slice(st * 128, (st + 1) * 128)
                            for nt in range(HID // 512):
                                nsl = slice(nt * 512, (nt + 1) * 512)
                                psa = pst.tile([128, 512], F32, tag="st")
                                nc.tensor.matmul(
                                    psa[:], ctxTz[:, 0, ssl], wd_sb[:, nsl],
                                    start=True, stop=True)
                                psb = pst.tile([128, 512], F32, tag="st")
                                nc.tensor.matmul(
                                    psb[:], ctxTz[:, 1, ssl], wd_sb[:, nsl],
                                    start=True, stop=True)
                                ob = osp.tile([128, 512], F32)
                                nc.vector.tensor_scalar_mul(
                                    ob[:], psa[:], denr[:, st, 0:1])
                                ob2 = osp.tile([128, 512], F32, tag="ob2")
                                nc.vector.scalar_tensor_tensor(
                                    ob2[:], psb[:], denr[:, st, 1:2], ob[:],
                                    op0=mybir.AluOpType.mult,
                                    op1=mybir.AluOpType.add)
                                nc.sync.dma_start(
                                    out[ssl, nsl], ob2[:])

    nc.compile()
    return nc


_NC_CACHE = None


def get_nc():
    global _NC_CACHE
    if _NC_CACHE is None:
        _NC_CACHE = build_nc()
    return _NC_CACHE


def make_in_maps(hidden_states, w_qkv, b_qkv, w_dense):
    hs = np.ascontiguousarray(
        np.asarray(hidden_states, dtype=np.float32).reshape(SEQ, HID))
    w_qkv = np.asarray(w_qkv, dtype=np.float32)
    b_qkv = np.asarray(b_qkv, dtype=np.float32)
    w_dense = np.asarray(w_dense, dtype=np.float32)
    # Reference layout: qkv.reshape(B, S, HEADS, 3*D) split on the last
    # axis, i.e. w_qkv columns are per-head [q_h | k_h | v_h] blocks of D.
    wq_cols = np.concatenate(
        [np.arange(h * 3 * D, h * 3 * D + D) for h in range(HEADS)])
    wk_cols = wq_cols + D
    wv_cols = wq_cols + 2 * D
    in_maps = []
    for c in range(NCORES):
        c0 = c * CW
        sel = slice(c0, c0 + CW)
        in_maps.append({
            "hs": hs,
            "wq": np.ascontiguousarray(w_qkv[:, wq_cols[sel]]),
            "wk": np.ascontiguousarray(w_qkv[:, wk_cols[sel]]),
            "wv": np.ascontiguousarray(w_qkv[:, wv_cols[sel]]),
            "bq": np.ascontiguousarray(b_qkv[wq_cols[sel]].reshape(CW, 1)),
            "bk": np.ascontiguousarray(b_qkv[wk_cols[sel]].reshape(CW, 1)),
            "wd": np.ascontiguousarray(w_dense[sel, :]),
        })
    return in_maps


def run(hidden_states, w_qkv, b_qkv, w_dense, b_dense, trace=False):
    nc = get_nc()
    in_maps = make_in_maps(hidden_states, w_qkv, b_qkv, w_dense)
    res = run_bass_kernel_spmd(nc, in_maps, core_ids=list(range(NCORES)),
                               trace=trace)
    acc = res.results[0]["out"].astype(np.float32)
    for c in range(1, NCORES):
        acc = acc + res.results[c]["out"]
    # bias terms that commute to the end: v-bias through dense, dense bias
    b_qkv = np.asarray(b_qkv, dtype=np.float32)
    b_v = np.concatenate(
        [b_qkv[h * 3 * D + 2 * D:h * 3 * D + 3 * D] for h in range(HEADS)])
    acc = acc + (b_v @ np.asarray(w_dense, dtype=np.float32)
                 + np.asarray(b_dense, dtype=np.float32))
    return acc.reshape(B, S, HID).astype(np.float32), res


def kernel(hidden_states, w_qkv, b_qkv, w_dense, b_dense):
    out, _ = run(hidden_states, w_qkv, b_qkv, w_dense, b_dense,
                 trace=bool(os.environ.get("BASS_TRACE")))
    return out


# revision 28
# speedup vs baseline: 1.4929x; 1.1598x over previous
"""Multi-head attention (B=2, S=2048, H=1024, 16 heads) on 8 NeuronCores.

Tensor-parallel sharding: 2 heads per core.  Each core computes QKV for its
heads, full attention over the sequence for its heads, and a partial output
projection (its 128 rows of w_dense).  The host sums the 8 partial outputs
(the all-reduce) and adds the output-side bias terms.

Layout notes (per core), all PE matmuls in plain 128x128 mode (mixing
64-row tiled and 128-row matmuls mode-thrashes the PE and halves its
clock, measured):
  hsT  [hid, seq]       hidden states transposed (PE transpose), streamed
                        in 512-seq windows.
  QTz/KTz [128, h, seq] q/k transposed per head, zero-padded to a full
                        128-partition contraction (rows 64-127 = 0).
  Vn  [128, 32, 2, 66]  v natural: partition = seq within 128-chunk,
                        [chunk, head, dim]; col 64 is 1.0 so the P@V
                        matmul also emits the softmax denominators.
  PT  [128, RING, 1024] exp(scores) ring: partition = k within chunk.
  ctxTz [128, h, seq]   unnormalized context transposed; head 0 in rows
                        0-63, head 1 in rows 64-127, other half zero, so
                        the dense matmul takes full w_dense slices.
  Softmax 1/sums are extracted by a basis-column matmul, reciprocals run
  wide on [128, 8], and the normalization lands in the dense epilogue as
  per-partition, per-head scalars.
"""

import os
import sys
import types

sys.path.insert(0, "/opt/trn_rl_repo")

import numpy as np


def _install_ntff_shim():
    """The trimmed container image lacks ``antenv.axon_hooks``, which
    ``run_bass_kernel_spmd(trace=True)`` needs to capture NTFF profiles
    under axon.  Recreate it from the boot helper + the injected .so."""
    if "antenv.axon_hooks" in sys.modules:
        return
    try:
        from trn_agent_boot.trn_boot import _ntff_profile_via_ctypes
        so = "/opt/axon/libaxon_pjrt.so"
        if not os.path.exists(so):
            return
        hook = _ntff_profile_via_ctypes(so)
        mod = types.ModuleType("antenv.axon_hooks")
        mod.get_axon_ntff_profile_hook = lambda: hook
        mod.set_axon_ntff_profile_hook = lambda h: None
        sys.modules["antenv.axon_hooks"] = mod
    except Exception:
        pass


_install_ntff_shim()

import concourse.bass as bass
import concourse.mybir as mybir
import concourse.tile as tile
from concourse import bacc
from concourse.bass_utils import run_bass_kernel_spmd
from concourse.masks import make_identity

F32 = mybir.dt.float32
F32R = mybir.dt.float32r
EXP = mybir.ActivationFunctionType.Exp

B, S, HID = 2, 2048, 1024
HEADS, D = 16, 64
SEQ = B * S                      # 4096 flattened rows
NCORES = 8
HPC = HEADS // NCORES            # heads per core = 2
CW = HPC * D                     # per-core width = 128
NHB = HID // 128                 # hidden 128-chunks = 8
WSEQ = 512                       # seq window for transpose+QKV
NWIN = SEQ // WSEQ               # 8
QW = 1024                        # q window in attention
NKT = S // 128                   # k chunks per batch = 16
NCH = SEQ // 128                 # global 128-row chunks = 32


def build_nc():
    nc = bacc.Bacc("TRN2", target_bir_lowering=False, debug=False,
                   num_devices=NCORES)

    hs = nc.dram_tensor("hs", [SEQ, HID], F32, kind="ExternalInput")
    wq = nc.dram_tensor("wq", [HID, CW], F32, kind="ExternalInput")
    wk = nc.dram_tensor("wk", [HID, CW], F32, kind="ExternalInput")
    wv = nc.dram_tensor("wv", [HID, CW], F32, kind="ExternalInput")
    bq = nc.dram_tensor("bq", [CW, 1], F32, kind="ExternalInput")
    bk = nc.dram_tensor("bk", [CW, 1], F32, kind="ExternalInput")
    wd = nc.dram_tensor("wd", [CW, HID], F32, kind="ExternalInput")
    out = nc.dram_tensor("out", [SEQ, HID], F32, kind="ExternalOutput")

    RING = 4

    with tile.TileContext(nc) as tc:
        with (
            tc.tile_pool(name="persist", bufs=1) as pp,
            tc.tile_pool(name="pt", bufs=1) as ptp,
        ):
            ident = pp.tile([128, 128], F32)
            make_identity(nc, ident[:])

            wq_sb = pp.tile([128, NHB, CW], F32R)
            wk_sb = pp.tile([128, NHB, CW], F32R)
            wv_sb = pp.tile([128, NHB, CW], F32R)
            for wsb, wdr in ((wq_sb, wq), (wk_sb, wk), (wv_sb, wv)):
                nc.sync.dma_start(
                    wsb[:],
                    wdr.ap().bitcast(F32R).rearrange("(c p) m -> p c m", p=128))
            bq_sb = pp.tile([CW, 1], F32)
            bk_sb = pp.tile([CW, 1], F32)
            nc.sync.dma_start(bq_sb[:], bq[:])
            nc.sync.dma_start(bk_sb[:], bk[:])
            wd_sb = pp.tile([CW, HID], F32R)
            nc.sync.dma_start(wd_sb[:], wd.ap().bitcast(F32R))

            # Everything on the PE stays in plain 128x128 mode.  Per-head
            # operands are zero-padded to a full 128-partition contraction:
            #   QTz/KTz [:, h, :]  rows 0-63 = head h, rows 64-127 = 0
            #   ctxTz   [:, 0, :]  rows 0-63 = head 0 ctx, upper rows 0
            #   ctxTz   [:, 1, :]  rows 64-127 = head 1 ctx, lower rows 0
            # so the dense matmul can take full-width w_dense slices.
            QTz = pp.tile([128, HPC, SEQ], F32R)
            KTz = pp.tile([128, HPC, SEQ], F32R)
            Vn = pp.tile([128, NCH, HPC, 66], F32R)
            ctxTz = pp.tile([128, HPC, SEQ], F32R)
            denr = pp.tile([128, NCH, HPC], F32)   # 1/rowsum, [q%128, st, h]
            den2 = pp.tile([128, HPC, QW], F32)    # row 0 = rowsums, rest 0
            PT = ptp.tile([128, RING, QW], F32R)

            nc.vector.memset(den2[:], 0.0)

            # ones column used by the P@V matmul to emit row sums
            ones_st = pp.tile([128, NCH * HPC], F32)
            nc.vector.memset(ones_st[:], 1.0)
            nc.vector.tensor_copy(
                Vn[:, :, :, 64:65],
                ones_st[:].rearrange("p (c h) -> p c h", c=NCH)
                .rearrange("p c h -> p c h ()"))

            # ---------------- phase 1: transpose hs + QKV projections ----
            with (
                tc.tile_pool(name="hsload", bufs=3) as hlp,
                tc.tile_pool(name="hstw", bufs=1) as hwp,
                tc.tile_pool(name="vtw", bufs=2) as vwp,
                tc.tile_pool(name="zs", bufs=1) as zsp,
                tc.tile_pool(name="ps_tr", bufs=2,
                             space=bass.MemorySpace.PSUM) as ptr,
                tc.tile_pool(name="ps_qkv", bufs=2,
                             space=bass.MemorySpace.PSUM) as pqk,
            ):
                # zero-fill the padded halves (f32r memset is rejected by
                # the ISA checker, so bounce through an f32 staging tile)
                zs = zsp.tile([D, SEQ], F32)
                nc.vector.memset(zs[:], 0.0)
                for h in range(HPC):
                    nc.vector.tensor_copy(QTz[D:128, h, :], zs[:])
                    nc.vector.tensor_copy(KTz[D:128, h, :], zs[:])
                nc.vector.tensor_copy(ctxTz[D:128, 0, :], zs[:])
                nc.vector.tensor_copy(ctxTz[0:D, 1, :], zs[:])

                hsT = hwp.tile([128, NHB, WSEQ], F32R)
                for w in range(NWIN):
                    r0 = w * WSEQ
                    for sb in range(WSEQ // 128):
                        hsn = hlp.tile([128, HID], F32)
                        nc.sync.dma_start(hsn[:], hs[r0 + sb * 128:
                                                     r0 + (sb + 1) * 128, :])
                        trp = ptr.tile([128, HID], F32, tag="tr")
                        for hb in range(NHB):
                            nc.tensor.transpose(
                                trp[:, hb * 128:(hb + 1) * 128],
                                hsn[:, hb * 128:(hb + 1) * 128],
                                ident[:])
                        if sb % 2 == 0:
                            nc.scalar.copy(
                                hsT[:, :, sb * 128:(sb + 1) * 128],
                                trp[:].rearrange("p (h s) -> p h s", h=NHB))
                        else:
                            nc.vector.tensor_copy(
                                hsT[:, :, sb * 128:(sb + 1) * 128],
                                trp[:].rearrange("p (h s) -> p h s", h=NHB))
                    for tgt in range(3):
                        ps = pqk.tile([128, WSEQ], F32, tag="qkv")
                        wsb = (wq_sb, wk_sb, wv_sb)[tgt]
                        for hb in range(NHB):
                            nc.tensor.matmul(
                                ps[:], wsb[:, hb, :], hsT[:, hb, :],
                                start=(hb == 0), stop=(hb == NHB - 1))
                        if tgt < 2:
                            dst, bias = ((QTz, bq_sb), (KTz, bk_sb))[tgt]
                            for h in range(HPC):
                                nc.vector.tensor_scalar_add(
                                    dst[0:D, h, r0:r0 + WSEQ],
                                    ps[h * D:(h + 1) * D, :],
                                    bias[h * D:(h + 1) * D, 0:1])
                        else:
                            vtw = vwp.tile([128, WSEQ], F32)
                            nc.vector.tensor_copy(vtw[:], ps[:])
                            vps = ptr.tile([128, WSEQ], F32, tag="vtr")
                            for sb2 in range(WSEQ // 128):
                                nc.tensor.transpose(
                                    vps[:, sb2 * 128:(sb2 + 1) * 128],
                                    vtw[:, sb2 * 128:(sb2 + 1) * 128],
                                    ident[:])
                            ch0 = r0 // 128
                            nc.vector.tensor_copy(
                                Vn[:, ch0:ch0 + 4, :, 0:64],
                                vps[:].rearrange("p (c h d) -> p c h d",
                                                 c=4, h=HPC))

            # ---------------- phase 2: attention + output projection -----
            with (
                tc.tile_pool(name="ps_st", bufs=2,
                             space=bass.MemorySpace.PSUM) as pst,
                tc.tile_pool(name="ps_pv", bufs=1,
                             space=bass.MemorySpace.PSUM) as ppv,
                tc.tile_pool(name="ps_dn", bufs=2,
                             space=bass.MemorySpace.PSUM) as pdn,
                tc.tile_pool(name="outst", bufs=3) as osp,
            ):
                for b in range(B):
                    for qw in range(S // QW):
                        qbase = b * S + qw * QW
                        qsl = slice(qbase, qbase + QW)
                        st0 = qbase // 128
                        for hh in range(HPC):
                            pvp = ppv.tile([D + 1, QW], F32, tag="pv")
                            for kt in range(NKT):
                                ch = b * NKT + kt
                                ksl = slice(b * S + kt * 128,
                                            b * S + (kt + 1) * 128)
                                rg = kt % RING
                                stp = pst.tile([128, QW], F32, tag="st")
                                for qh in range(QW // 512):
                                    sl = slice(qh * 512, (qh + 1) * 512)
                                    nc.tensor.matmul(
                                        stp[:, sl], KTz[:, hh, ksl],
                                        QTz[:, hh,
                                            qbase + qh * 512:
                                            qbase + (qh + 1) * 512],
                                        start=True, stop=True)
                                nc.scalar.activation(
                                    PT[:, rg, :], stp[:], EXP, scale=0.125)
                                for qh in range(QW // 512):
                                    sl = slice(qh * 512, (qh + 1) * 512)
                                    nc.tensor.matmul(
                                        pvp[:, sl], Vn[:, ch, hh, 0:65],
                                        PT[:, rg, sl],
                                        start=(kt == 0),
                                        stop=(kt == NKT - 1))
                            # ctx into its head's partition range; keep the
                            # denominator row for the normalization pass
                            nc.vector.tensor_copy(
                                ctxTz[hh * D:(hh + 1) * D, hh, qsl],
                                pvp[0:D, :])
                            nc.scalar.copy(
                                den2[0:1, hh, :], pvp[D:D + 1, :])
                            # transpose the rowsum row via basis column
                            dnp = pdn.tile([128, QW // 128], F32, tag="dn")
                            for qt in range(QW // 128):
                                nc.tensor.matmul(
                                    dnp[:, qt:qt + 1],
                                    den2[:, hh, qt * 128:(qt + 1) * 128],
                                    ident[:, 0:1],
                                    start=True, stop=True)
                            nc.vector.reciprocal(
                                denr[:, st0:st0 + QW // 128, hh:hh + 1],
                                dnp[:])
                        # dense partial; per-head normalization as
                        # per-partition scalars in the epilogue
                        for stl in range(QW // 128):
                            st = qbase // 128 + stl
                            ssl = slice(st * 128, (st + 1) * 128)
                            for nt in range(HID // 512):
                                nsl = slice(nt * 512, (nt + 1) * 512)
                                psa = pdn.tile([128, 512], F32, tag="dn")
                                nc.tensor.matmul(
                                    psa[:], ctxTz[:, 0, ssl], wd_sb[:, nsl],
                                    start=True, stop=True)
                                psb = pdn.tile([128, 512], F32, tag="dn")
                                nc.tensor.matmul(
                                    psb[:], ctxTz[:, 1, ssl], wd_sb[:, nsl],
                                    start=True, stop=True)
                                ob = osp.tile([128, 512], F32)
                                nc.vector.tensor_scalar_mul(
                                    ob[:], psa[:], denr[:, st, 0:1])
                                ob2 = osp.tile([128, 512], F32, tag="ob2")
                                nc.vector.scalar_tensor_tensor(
                                    ob2[:], psb[:], denr[:, st, 1:2], ob[:],
                                    op0=mybir.AluOpType.mult,
                                    op1=mybir.AluOpType.add)
                                nc.sync.dma_start(
                                    out[ssl, nsl], ob2[:])

    nc.compile()
    return nc


_NC_CACHE = None


def get_nc():
    global _NC_CACHE
    if _NC_CACHE is None:
        _NC_CACHE = build_nc()
    return _NC_CACHE


def make_in_maps(hidden_states, w_qkv, b_qkv, w_dense):
    hs = np.ascontiguousarray(
        np.asarray(hidden_states, dtype=np.float32).reshape(SEQ, HID))
    w_qkv = np.asarray(w_qkv, dtype=np.float32)
    b_qkv = np.asarray(b_qkv, dtype=np.float32)
    w_dense = np.asarray(w_dense, dtype=np.float32)
    # Reference layout: qkv.reshape(B, S, HEADS, 3*D) split on the last
    # axis, i.e. w_qkv columns are per-head [q_h | k_h | v_h] blocks of D.
    wq_cols = np.concatenate(
        [np.arange(h * 3 * D, h * 3 * D + D) for h in range(HEADS)])
    wk_cols = wq_cols + D
    wv_cols = wq_cols + 2 * D
    in_maps = []
    for c in range(NCORES):
        c0 = c * CW
        sel = slice(c0, c0 + CW)
        in_maps.append({
            "hs": hs,
            "wq": np.ascontiguousarray(w_qkv[:, wq_cols[sel]]),
            "wk": np.ascontiguousarray(w_qkv[:, wk_cols[sel]]),
            "wv": np.ascontiguousarray(w_qkv[:, wv_cols[sel]]),
            "bq": np.ascontiguousarray(b_qkv[wq_cols[sel]].reshape(CW, 1)),
            "bk": np.ascontiguousarray(b_qkv[wk_cols[sel]].reshape(CW, 1)),
            "wd": np.ascontiguousarray(w_dense[sel, :]),
        })
    return in_maps


def run(hidden_states, w_qkv, b_qkv, w_dense, b_dense, trace=False):
    nc = get_nc()
    in_maps = make_in_maps(hidden_states, w_qkv, b_qkv, w_dense)
    res = run_bass_kernel_spmd(nc, in_maps, core_ids=list(range(NCORES)),
                               trace=trace)
    acc = res.results[0]["out"].astype(np.float32)
    for c in range(1, NCORES):
        acc = acc + res.results[c]["out"]
    # bias terms that commute to the end: v-bias through dense, dense bias
    b_qkv = np.asarray(b_qkv, dtype=np.float32)
    b_v = np.concatenate(
        [b_qkv[h * 3 * D + 2 * D:h * 3 * D + 3 * D] for h in range(HEADS)])
    acc = acc + (b_v @ np.asarray(w_dense, dtype=np.float32)
                 + np.asarray(b_dense, dtype=np.float32))
    return acc.reshape(B, S, HID).astype(np.float32), res


def kernel(hidden_states, w_qkv, b_qkv, w_dense, b_dense):
    out, _ = run(hidden_states, w_qkv, b_qkv, w_dense, b_dense,
                 trace=bool(os.environ.get("BASS_TRACE")))
    return out


# revision 29
# speedup vs baseline: 1.6266x; 1.0895x over previous
"""Multi-head attention (B=2, S=2048, H=1024, 16 heads) on 8 NeuronCores.

Tensor-parallel sharding: 2 heads per core.  Each core computes QKV for its
heads, full attention over the sequence for its heads, and a partial output
projection (its 128 rows of w_dense).  The host sums the 8 partial outputs
(the all-reduce) and adds the output-side bias terms.

Layout notes (per core), all PE matmuls in plain 128x128 mode (mixing
64-row tiled and 128-row matmuls mode-thrashes the PE and halves its
clock, measured):
  hsT  [hid, seq]       hidden states transposed (PE transpose), streamed
                        in 512-seq windows.
  QTz/KTz [128, h, seq] q/k transposed per head, zero-padded to a full
                        128-partition contraction (rows 64-127 = 0).
  Vn  [128, 32, 2, 66]  v natural: partition = seq within 128-chunk,
                        [chunk, head, dim]; col 64 is 1.0 so the P@V
                        matmul also emits the softmax denominators.
  PT  [128, RING, 1024] exp(scores) ring: partition = k within chunk.
  ctxTz [128, h, seq]   unnormalized context transposed; head 0 in rows
                        0-63, head 1 in rows 64-127, other half zero, so
                        the dense matmul takes full w_dense slices.
  Softmax 1/sums are extracted by a basis-column matmul, reciprocals run
  wide on [128, 8], and the normalization lands in the dense epilogue as
  per-partition, per-head scalars.
"""

import os
import sys
import types

sys.path.insert(0, "/opt/trn_rl_repo")

import numpy as np


def _install_ntff_shim():
    """The trimmed container image lacks ``antenv.axon_hooks``, which
    ``run_bass_kernel_spmd(trace=True)`` needs to capture NTFF profiles
    under axon.  Recreate it from the boot helper + the injected .so."""
    if "antenv.axon_hooks" in sys.modules:
        return
    try:
        from trn_agent_boot.trn_boot import _ntff_profile_via_ctypes
        so = "/opt/axon/libaxon_pjrt.so"
        if not os.path.exists(so):
            return
        hook = _ntff_profile_via_ctypes(so)
        mod = types.ModuleType("antenv.axon_hooks")
        mod.get_axon_ntff_profile_hook = lambda: hook
        mod.set_axon_ntff_profile_hook = lambda h: None
        sys.modules["antenv.axon_hooks"] = mod
    except Exception:
        pass


_install_ntff_shim()

import concourse.bass as bass
import concourse.mybir as mybir
import concourse.tile as tile
from concourse import bacc
from concourse.bass_utils import run_bass_kernel_spmd
from concourse.masks import make_identity

F32 = mybir.dt.float32
F32R = mybir.dt.float32r
EXP = mybir.ActivationFunctionType.Exp

B, S, HID = 2, 2048, 1024
HEADS, D = 16, 64
SEQ = B * S                      # 4096 flattened rows
NCORES = 8
HPC = HEADS // NCORES            # heads per core = 2
CW = HPC * D                     # per-core width = 128
NHB = HID // 128                 # hidden 128-chunks = 8
WSEQ = 512                       # seq window for transpose+QKV
NWIN = SEQ // WSEQ               # 8
QW = 1024                        # q window in attention
NKT = S // 128                   # k chunks per batch = 16
NCH = SEQ // 128                 # global 128-row chunks = 32


def build_nc():
    nc = bacc.Bacc("TRN2", target_bir_lowering=False, debug=False,
                   num_devices=NCORES)

    hs = nc.dram_tensor("hs", [SEQ, HID], F32, kind="ExternalInput")
    wq = nc.dram_tensor("wq", [HID, CW], F32, kind="ExternalInput")
    wk = nc.dram_tensor("wk", [HID, CW], F32, kind="ExternalInput")
    wv = nc.dram_tensor("wv", [HID, CW], F32, kind="ExternalInput")
    bq = nc.dram_tensor("bq", [CW, 1], F32, kind="ExternalInput")
    bk = nc.dram_tensor("bk", [CW, 1], F32, kind="ExternalInput")
    wd = nc.dram_tensor("wd", [CW, HID], F32, kind="ExternalInput")
    out = nc.dram_tensor("out", [SEQ, HID], F32, kind="ExternalOutput")

    RING = 4

    with tile.TileContext(nc) as tc:
        with (
            tc.tile_pool(name="persist", bufs=1) as pp,
            tc.tile_pool(name="pt", bufs=1) as ptp,
        ):
            ident = pp.tile([128, 128], F32)
            make_identity(nc, ident[:])

            wq_sb = pp.tile([128, NHB, CW], F32R)
            wk_sb = pp.tile([128, NHB, CW], F32R)
            wv_sb = pp.tile([128, NHB, CW], F32R)
            for wsb, wdr in ((wq_sb, wq), (wk_sb, wk), (wv_sb, wv)):
                nc.sync.dma_start(
                    wsb[:],
                    wdr.ap().bitcast(F32R).rearrange("(c p) m -> p c m", p=128))
            bq_sb = pp.tile([CW, 1], F32)
            bk_sb = pp.tile([CW, 1], F32)
            nc.sync.dma_start(bq_sb[:], bq[:])
            nc.sync.dma_start(bk_sb[:], bk[:])
            wd_sb = pp.tile([CW, HID], F32R)
            nc.sync.dma_start(wd_sb[:], wd.ap().bitcast(F32R))

            # Everything on the PE stays in plain 128x128 mode.  Per-head
            # operands are zero-padded to a full 128-partition contraction:
            #   QTz/KTz [:, h, :]  rows 0-63 = head h, rows 64-127 = 0
            #   ctxTz   [:, 0, :]  rows 0-63 = head 0 ctx, upper rows 0
            #   ctxTz   [:, 1, :]  rows 64-127 = head 1 ctx, lower rows 0
            # so the dense matmul can take full-width w_dense slices.
            QTz = pp.tile([128, HPC, SEQ], F32R)
            KTz = pp.tile([128, HPC, SEQ], F32R)
            Vn = pp.tile([128, NCH, HPC, 66], F32R)
            ctxTz = pp.tile([128, HPC, SEQ], F32R)
            denr = pp.tile([128, NCH, HPC], F32)   # 1/rowsum, [q%128, st, h]
            den2 = pp.tile([128, HPC, QW], F32)    # row 0 = rowsums, rest 0
            PT = ptp.tile([128, RING, QW], F32R)

            nc.vector.memset(den2[:], 0.0)

            # ones column used by the P@V matmul to emit row sums
            ones_st = pp.tile([128, NCH * HPC], F32)
            nc.vector.memset(ones_st[:], 1.0)
            nc.vector.tensor_copy(
                Vn[:, :, :, 64:65],
                ones_st[:].rearrange("p (c h) -> p c h", c=NCH)
                .rearrange("p c h -> p c h ()"))

            # ---------------- phase 1: transpose hs + QKV projections ----
            with (
                tc.tile_pool(name="hsload", bufs=4) as hlp,
                tc.tile_pool(name="hstw", bufs=1) as hwp,
                tc.tile_pool(name="vtw", bufs=2) as vwp,
                tc.tile_pool(name="zs", bufs=1) as zsp,
                tc.tile_pool(name="ps_tr", bufs=2,
                             space=bass.MemorySpace.PSUM) as ptr,
                tc.tile_pool(name="ps_qkv", bufs=2,
                             space=bass.MemorySpace.PSUM) as pqk,
            ):
                # zero-fill the padded halves (f32r memset is rejected by
                # the ISA checker, so bounce through an f32 staging tile)
                zs = zsp.tile([D, SEQ], F32)
                nc.vector.memset(zs[:], 0.0)
                for h in range(HPC):
                    nc.gpsimd.tensor_copy(QTz[D:128, h, :], zs[:])
                    nc.gpsimd.tensor_copy(KTz[D:128, h, :], zs[:])
                nc.gpsimd.tensor_copy(ctxTz[D:128, 0, :], zs[:])
                nc.gpsimd.tensor_copy(ctxTz[0:D, 1, :], zs[:])

                hsT = hwp.tile([128, NHB, WSEQ], F32R)
                for w in range(NWIN):
                    r0 = w * WSEQ
                    for sb in range(WSEQ // 128):
                        hsn = hlp.tile([128, HID], F32)
                        nc.sync.dma_start(hsn[:], hs[r0 + sb * 128:
                                                     r0 + (sb + 1) * 128, :])
                        trp = ptr.tile([128, HID], F32, tag="tr")
                        for hb in range(NHB):
                            nc.tensor.transpose(
                                trp[:, hb * 128:(hb + 1) * 128],
                                hsn[:, hb * 128:(hb + 1) * 128],
                                ident[:])
                        if sb % 2 == 0:
                            nc.scalar.copy(
                                hsT[:, :, sb * 128:(sb + 1) * 128],
                                trp[:].rearrange("p (h s) -> p h s", h=NHB))
                        else:
                            nc.vector.tensor_copy(
                                hsT[:, :, sb * 128:(sb + 1) * 128],
                                trp[:].rearrange("p (h s) -> p h s", h=NHB))
                    for tgt in range(3):
                        ps = pqk.tile([128, WSEQ], F32, tag="qkv")
                        wsb = (wq_sb, wk_sb, wv_sb)[tgt]
                        for hb in range(NHB):
                            nc.tensor.matmul(
                                ps[:], wsb[:, hb, :], hsT[:, hb, :],
                                start=(hb == 0), stop=(hb == NHB - 1))
                        if tgt < 2:
                            dst, bias = ((QTz, bq_sb), (KTz, bk_sb))[tgt]
                            for h in range(HPC):
                                nc.vector.tensor_scalar_add(
                                    dst[0:D, h, r0:r0 + WSEQ],
                                    ps[h * D:(h + 1) * D, :],
                                    bias[h * D:(h + 1) * D, 0:1])
                        else:
                            vtw = vwp.tile([128, WSEQ], F32)
                            nc.vector.tensor_copy(vtw[:], ps[:])
                            vps = ptr.tile([128, WSEQ], F32, tag="vtr")
                            for sb2 in range(WSEQ // 128):
                                nc.tensor.transpose(
                                    vps[:, sb2 * 128:(sb2 + 1) * 128],
                                    vtw[:, sb2 * 128:(sb2 + 1) * 128],
                                    ident[:])
                            ch0 = r0 // 128
                            nc.vector.tensor_copy(
                                Vn[:, ch0:ch0 + 4, :, 0:64],
                                vps[:].rearrange("p (c h d) -> p c h d",
                                                 c=4, h=HPC))

            # ---------------- phase 2: attention + output projection -----
            with (
                tc.tile_pool(name="ps_st", bufs=2,
                             space=bass.MemorySpace.PSUM) as pst,
                tc.tile_pool(name="ps_pv", bufs=1,
                             space=bass.MemorySpace.PSUM) as ppv,
                tc.tile_pool(name="ps_dn", bufs=2,
                             space=bass.MemorySpace.PSUM) as pdn,
                tc.tile_pool(name="outst", bufs=4) as osp,
            ):
                for b in range(B):
                    for qw in range(S // QW):
                        qbase = b * S + qw * QW
                        qsl = slice(qbase, qbase + QW)
                        st0 = qbase // 128
                        for hh in range(HPC):
                            pvp = ppv.tile([D + 1, QW], F32, tag="pv")
                            for kt in range(NKT):
                                ch = b * NKT + kt
                                ksl = slice(b * S + kt * 128,
                                            b * S + (kt + 1) * 128)
                                rg = kt % RING
                                stp = pst.tile([128, QW], F32, tag="st")
                                for qh in range(QW // 512):
                                    sl = slice(qh * 512, (qh + 1) * 512)
                                    nc.tensor.matmul(
                                        stp[:, sl], KTz[:, hh, ksl],
                                        QTz[:, hh,
                                            qbase + qh * 512:
                                            qbase + (qh + 1) * 512],
                                        start=True, stop=True)
                                nc.scalar.activation(
                                    PT[:, rg, :], stp[:], EXP, scale=0.125)
                                for qh in range(QW // 512):
                                    sl = slice(qh * 512, (qh + 1) * 512)
                                    nc.tensor.matmul(
                                        pvp[:, sl], Vn[:, ch, hh, 0:65],
                                        PT[:, rg, sl],
                                        start=(kt == 0),
                                        stop=(kt == NKT - 1))
                            # ctx into its head's partition range; keep the
                            # denominator row for the normalization pass
                            nc.vector.tensor_copy(
                                ctxTz[hh * D:(hh + 1) * D, hh, qsl],
                                pvp[0:D, :])
                            nc.scalar.copy(
                                den2[0:1, hh, :], pvp[D:D + 1, :])
                            # transpose the rowsum row via basis column
                            dnp = pdn.tile([128, QW // 128], F32, tag="dn")
                            for qt in range(QW // 128):
                                nc.tensor.matmul(
                                    dnp[:, qt:qt + 1],
                                    den2[:, hh, qt * 128:(qt + 1) * 128],
                                    ident[:, 0:1],
                                    start=True, stop=True)
                            nc.vector.reciprocal(
                                denr[:, st0:st0 + QW // 128, hh:hh + 1],
                                dnp[:])
                        # dense partial; per-head normalization as
                        # per-partition scalars in the epilogue
                        for stl in range(QW // 128):
                            st = qbase // 128 + stl
                            ssl = slice(st * 128, (st + 1) * 128)
                            for nt in range(HID // 512):
                                nsl = slice(nt * 512, (nt + 1) * 512)
                                psa = pdn.tile([128, 512], F32, tag="dn")
                                nc.tensor.matmul(
                                    psa[:], ctxTz[:, 0, ssl], wd_sb[:, nsl],
                                    start=True, stop=True)
                                psb = pdn.tile([128, 512], F32, tag="dn")
                                nc.tensor.matmul(
                                    psb[:], ctxTz[:, 1, ssl], wd_sb[:, nsl],
                                    start=True, stop=True)
                                ob = osp.tile([128, 512], F32)
                                nc.vector.tensor_scalar_mul(
                                    ob[:], psa[:], denr[:, st, 0:1])
                                ob2 = osp.tile([128, 512], F32, tag="ob2")
                                nc.vector.scalar_tensor_tensor(
                                    ob2[:], psb[:], denr[:, st, 1:2], ob[:],
                                    op0=mybir.AluOpType.mult,
                                    op1=mybir.AluOpType.add)
                                nc.sync.dma_start(
                                    out[ssl, nsl], ob2[:])

    nc.compile()
    return nc


_NC_CACHE = None


def get_nc():
    global _NC_CACHE
    if _NC_CACHE is None:
        _NC_CACHE = build_nc()
    return _NC_CACHE


def make_in_maps(hidden_states, w_qkv, b_qkv, w_dense):
    hs = np.ascontiguousarray(
        np.asarray(hidden_states, dtype=np.float32).reshape(SEQ, HID))
    w_qkv = np.asarray(w_qkv, dtype=np.float32)
    b_qkv = np.asarray(b_qkv, dtype=np.float32)
    w_dense = np.asarray(w_dense, dtype=np.float32)
    # Reference layout: qkv.reshape(B, S, HEADS, 3*D) split on the last
    # axis, i.e. w_qkv columns are per-head [q_h | k_h | v_h] blocks of D.
    wq_cols = np.concatenate(
        [np.arange(h * 3 * D, h * 3 * D + D) for h in range(HEADS)])
    wk_cols = wq_cols + D
    wv_cols = wq_cols + 2 * D
    in_maps = []
    for c in range(NCORES):
        c0 = c * CW
        sel = slice(c0, c0 + CW)
        in_maps.append({
            "hs": hs,
            "wq": np.ascontiguousarray(w_qkv[:, wq_cols[sel]]),
            "wk": np.ascontiguousarray(w_qkv[:, wk_cols[sel]]),
            "wv": np.ascontiguousarray(w_qkv[:, wv_cols[sel]]),
            "bq": np.ascontiguousarray(b_qkv[wq_cols[sel]].reshape(CW, 1)),
            "bk": np.ascontiguousarray(b_qkv[wk_cols[sel]].reshape(CW, 1)),
            "wd": np.ascontiguousarray(w_dense[sel, :]),
        })
    return in_maps


def run(hidden_states, w_qkv, b_qkv, w_dense, b_dense, trace=False):
    nc = get_nc()
    in_maps = make_in_maps(hidden_states, w_qkv, b_qkv, w_dense)
    res = run_bass_kernel_spmd(nc, in_maps, core_ids=list(range(NCORES)),
                               trace=trace)
    acc = res.results[0]["out"].astype(np.float32)
    for c in range(1, NCORES):
        acc = acc + res.results[c]["out"]
    # bias terms that commute to the end: v-bias through dense, dense bias
    b_qkv = np.asarray(b_qkv, dtype=np.float32)
    b_v = np.concatenate(
        [b_qkv[h * 3 * D + 2 * D:h * 3 * D + 3 * D] for h in range(HEADS)])
    acc = acc + (b_v @ np.asarray(w_dense, dtype=np.float32)
                 + np.asarray(b_dense, dtype=np.float32))
    return acc.reshape(B, S, HID).astype(np.float32), res


def kernel(hidden_states, w_qkv, b_qkv, w_dense, b_dense):
    out, _ = run(hidden_states, w_qkv, b_qkv, w_dense, b_dense,
                 trace=bool(os.environ.get("BASS_TRACE")))
    return out


# revision 30
# speedup vs baseline: 1.6577x; 1.0192x over previous
"""Multi-head attention (B=2, S=2048, H=1024, 16 heads) on 8 NeuronCores.

Tensor-parallel sharding: 2 heads per core.  Each core computes QKV for its
heads, full attention over the sequence for its heads, and a partial output
projection (its 128 rows of w_dense).  The host sums the 8 partial outputs
(the all-reduce) and adds the output-side bias terms.

Layout notes (per core), all PE matmuls in plain 128x128 mode (mixing
64-row tiled and 128-row matmuls mode-thrashes the PE and halves its
clock, measured):
  hsT  [hid, seq]       hidden states transposed (PE transpose), streamed
                        in 512-seq windows.
  QTz/KTz [128, h, seq] q/k transposed per head, zero-padded to a full
                        128-partition contraction (rows 64-127 = 0).
  Vn  [128, 32, 2, 66]  v natural: partition = seq within 128-chunk,
                        [chunk, head, dim]; col 64 is 1.0 so the P@V
                        matmul also emits the softmax denominators.
  PT  [128, RING, 1024] exp(scores) ring: partition = k within chunk.
  ctxTz [128, h, seq]   unnormalized context transposed; head 0 in rows
                        0-63, head 1 in rows 64-127, other half zero, so
                        the dense matmul takes full w_dense slices.
  Softmax 1/sums are extracted by a basis-column matmul, reciprocals run
  wide on [128, 8], and the normalization lands in the dense epilogue as
  per-partition, per-head scalars.
"""

import os
import sys
import types

sys.path.insert(0, "/opt/trn_rl_repo")

import numpy as np


def _install_ntff_shim():
    """The trimmed container image lacks ``antenv.axon_hooks``, which
    ``run_bass_kernel_spmd(trace=True)`` needs to capture NTFF profiles
    under axon.  Recreate it from the boot helper + the injected .so."""
    if "antenv.axon_hooks" in sys.modules:
        return
    try:
        from trn_agent_boot.trn_boot import _ntff_profile_via_ctypes
        so = "/opt/axon/libaxon_pjrt.so"
        if not os.path.exists(so):
            return
        hook = _ntff_profile_via_ctypes(so)
        mod = types.ModuleType("antenv.axon_hooks")
        mod.get_axon_ntff_profile_hook = lambda: hook
        mod.set_axon_ntff_profile_hook = lambda h: None
        sys.modules["antenv.axon_hooks"] = mod
    except Exception:
        pass


_install_ntff_shim()

import concourse.bass as bass
import concourse.mybir as mybir
import concourse.tile as tile
from concourse import bacc
from concourse.bass_utils import run_bass_kernel_spmd
from concourse.masks import make_identity

F32 = mybir.dt.float32
F32R = mybir.dt.float32r
EXP = mybir.ActivationFunctionType.Exp

B, S, HID = 2, 2048, 1024
HEADS, D = 16, 64
SEQ = B * S                      # 4096 flattened rows
NCORES = 8
HPC = HEADS // NCORES            # heads per core = 2
CW = HPC * D                     # per-core width = 128
NHB = HID // 128                 # hidden 128-chunks = 8
WSEQ = 512                       # seq window for transpose+QKV
NWIN = SEQ // WSEQ               # 8
QW = 1024                        # q window in attention
NKT = S // 128                   # k chunks per batch = 16
NCH = SEQ // 128                 # global 128-row chunks = 32


def build_nc():
    nc = bacc.Bacc("TRN2", target_bir_lowering=False, debug=False,
                   num_devices=NCORES)

    hs = nc.dram_tensor("hs", [SEQ, HID], F32, kind="ExternalInput")
    wq = nc.dram_tensor("wq", [HID, CW], F32, kind="ExternalInput")
    wk = nc.dram_tensor("wk", [HID, CW], F32, kind="ExternalInput")
    wv = nc.dram_tensor("wv", [HID, CW], F32, kind="ExternalInput")
    bq = nc.dram_tensor("bq", [CW, 1], F32, kind="ExternalInput")
    bk = nc.dram_tensor("bk", [CW, 1], F32, kind="ExternalInput")
    wd = nc.dram_tensor("wd", [CW, HID], F32, kind="ExternalInput")
    out = nc.dram_tensor("out", [SEQ, HID], F32, kind="ExternalOutput")

    RING = 4

    with tile.TileContext(nc) as tc:
        with (
            tc.tile_pool(name="persist", bufs=1) as pp,
            tc.tile_pool(name="pt", bufs=1) as ptp,
        ):
            ident = pp.tile([128, 128], F32)
            make_identity(nc, ident[:])

            wq_sb = pp.tile([128, NHB, CW], F32R)
            wk_sb = pp.tile([128, NHB, CW], F32R)
            wv_sb = pp.tile([128, NHB, CW], F32R)
            for wsb, wdr in ((wq_sb, wq), (wk_sb, wk), (wv_sb, wv)):
                nc.sync.dma_start(
                    wsb[:],
                    wdr.ap().bitcast(F32R).rearrange("(c p) m -> p c m", p=128))
            bq_sb = pp.tile([CW, 1], F32)
            bk_sb = pp.tile([CW, 1], F32)
            nc.sync.dma_start(bq_sb[:], bq[:])
            nc.sync.dma_start(bk_sb[:], bk[:])
            wd_sb = pp.tile([CW, HID], F32R)
            nc.sync.dma_start(wd_sb[:], wd.ap().bitcast(F32R))

            # Everything on the PE stays in plain 128x128 mode.  Per-head
            # operands are zero-padded to a full 128-partition contraction:
            #   QTz/KTz [:, h, :]  rows 0-63 = head h, rows 64-127 = 0
            #   ctxTz   [:, 0, :]  rows 0-63 = head 0 ctx, upper rows 0
            #   ctxTz   [:, 1, :]  rows 64-127 = head 1 ctx, lower rows 0
            # so the dense matmul can take full-width w_dense slices.
            QTz = pp.tile([128, HPC, SEQ], F32R)
            KTz = pp.tile([128, HPC, SEQ], F32R)
            Vn = pp.tile([128, NCH, HPC, 66], F32R)
            ctxTz = pp.tile([128, HPC, SEQ], F32R)
            denr = pp.tile([128, NCH, HPC], F32)   # 1/rowsum, [q%128, st, h]
            den2 = pp.tile([128, QW], F32)  # rows 0/64 = h0/h1 rowsums
            PT = ptp.tile([128, RING, QW], F32R)

            nc.vector.memset(den2[:], 0.0)

            # ones column used by the P@V matmul to emit row sums
            ones_st = pp.tile([128, NCH * HPC], F32)
            nc.vector.memset(ones_st[:], 1.0)
            nc.vector.tensor_copy(
                Vn[:, :, :, 64:65],
                ones_st[:].rearrange("p (c h) -> p c h", c=NCH)
                .rearrange("p c h -> p c h ()"))

            # ---------------- phase 1: transpose hs + QKV projections ----
            with (
                tc.tile_pool(name="hsload", bufs=4) as hlp,
                tc.tile_pool(name="hstw", bufs=1) as hwp,
                tc.tile_pool(name="vtw", bufs=2) as vwp,
                tc.tile_pool(name="zs", bufs=1) as zsp,
                tc.tile_pool(name="ps_tr", bufs=2,
                             space=bass.MemorySpace.PSUM) as ptr,
                tc.tile_pool(name="ps_qkv", bufs=2,
                             space=bass.MemorySpace.PSUM) as pqk,
            ):
                # zero-fill the padded halves (f32r memset is rejected by
                # the ISA checker, so bounce through an f32 staging tile)
                zs = zsp.tile([D, SEQ], F32)
                nc.vector.memset(zs[:], 0.0)
                for h in range(HPC):
                    nc.gpsimd.tensor_copy(QTz[D:128, h, :], zs[:])
                    nc.gpsimd.tensor_copy(KTz[D:128, h, :], zs[:])
                nc.gpsimd.tensor_copy(ctxTz[D:128, 0, :], zs[:])
                nc.gpsimd.tensor_copy(ctxTz[0:D, 1, :], zs[:])

                hsT = hwp.tile([128, NHB, WSEQ], F32R)
                for w in range(NWIN):
                    r0 = w * WSEQ
                    for sb in range(WSEQ // 128):
                        hsn = hlp.tile([128, HID], F32)
                        nc.sync.dma_start(hsn[:], hs[r0 + sb * 128:
                                                     r0 + (sb + 1) * 128, :])
                        trp = ptr.tile([128, HID], F32, tag="tr")
                        for hb in range(NHB):
                            nc.tensor.transpose(
                                trp[:, hb * 128:(hb + 1) * 128],
                                hsn[:, hb * 128:(hb + 1) * 128],
                                ident[:])
                        if sb % 2 == 0:
                            nc.scalar.copy(
                                hsT[:, :, sb * 128:(sb + 1) * 128],
                                trp[:].rearrange("p (h s) -> p h s", h=NHB))
                        else:
                            nc.vector.tensor_copy(
                                hsT[:, :, sb * 128:(sb + 1) * 128],
                                trp[:].rearrange("p (h s) -> p h s", h=NHB))
                    for tgt in range(3):
                        ps = pqk.tile([128, WSEQ], F32, tag="qkv")
                        wsb = (wq_sb, wk_sb, wv_sb)[tgt]
                        for hb in range(NHB):
                            nc.tensor.matmul(
                                ps[:], wsb[:, hb, :], hsT[:, hb, :],
                                start=(hb == 0), stop=(hb == NHB - 1))
                        if tgt < 2:
                            dst, bias = ((QTz, bq_sb), (KTz, bk_sb))[tgt]
                            for h in range(HPC):
                                nc.vector.tensor_scalar_add(
                                    dst[0:D, h, r0:r0 + WSEQ],
                                    ps[h * D:(h + 1) * D, :],
                                    bias[h * D:(h + 1) * D, 0:1])
                        else:
                            vtw = vwp.tile([128, WSEQ], F32)
                            nc.vector.tensor_copy(vtw[:], ps[:])
                            vps = ptr.tile([128, WSEQ], F32, tag="vtr")
                            for sb2 in range(WSEQ // 128):
                                nc.tensor.transpose(
                                    vps[:, sb2 * 128:(sb2 + 1) * 128],
                                    vtw[:, sb2 * 128:(sb2 + 1) * 128],
                                    ident[:])
                            ch0 = r0 // 128
                            nc.vector.tensor_copy(
                                Vn[:, ch0:ch0 + 4, :, 0:64],
                                vps[:].rearrange("p (c h d) -> p c h d",
                                                 c=4, h=HPC))

            # ---------------- phase 2: attention + output projection -----
            with (
                tc.tile_pool(name="ps_st", bufs=2,
                             space=bass.MemorySpace.PSUM) as pst,
                tc.tile_pool(name="ps_pv", bufs=1,
                             space=bass.MemorySpace.PSUM) as ppv,
                tc.tile_pool(name="ps_dn", bufs=2,
                             space=bass.MemorySpace.PSUM) as pdn,
                tc.tile_pool(name="outst", bufs=4) as osp,
            ):
                for b in range(B):
                    for qw in range(S // QW):
                        qbase = b * S + qw * QW
                        qsl = slice(qbase, qbase + QW)
                        st0 = qbase // 128
                        for hh in range(HPC):
                            pvp = ppv.tile([D + 1, QW], F32, tag="pv")
                            for kt in range(NKT):
                                ch = b * NKT + kt
                                ksl = slice(b * S + kt * 128,
                                            b * S + (kt + 1) * 128)
                                rg = kt % RING
                                stp = pst.tile([128, QW], F32, tag="st")
                                for qh in range(QW // 512):
                                    sl = slice(qh * 512, (qh + 1) * 512)
                                    nc.tensor.matmul(
                                        stp[:, sl], KTz[:, hh, ksl],
                                        QTz[:, hh,
                                            qbase + qh * 512:
                                            qbase + (qh + 1) * 512],
                                        start=True, stop=True)
                                nc.scalar.activation(
                                    PT[:, rg, :], stp[:], EXP, scale=0.125)
                                for qh in range(QW // 512):
                                    sl = slice(qh * 512, (qh + 1) * 512)
                                    nc.tensor.matmul(
                                        pvp[:, sl], Vn[:, ch, hh, 0:65],
                                        PT[:, rg, sl],
                                        start=(kt == 0),
                                        stop=(kt == NKT - 1))
                            # ctx into its head's partition range; keep the
                            # denominator row for the normalization pass
                            nc.vector.tensor_copy(
                                ctxTz[hh * D:(hh + 1) * D, hh, qsl],
                                pvp[0:D, :])
                            if hh == 0:
                                nc.scalar.copy(
                                    den2[0:1, :], pvp[D:D + 1, :])
                            else:
                                nc.vector.tensor_copy(
                                    den2[64:65, :], pvp[D:D + 1, :])
                        # extract both heads' rowsum rows transposed in one
                        # matmul per 128-q block via basis columns 0 and 64
                        dnp = pdn.tile([128, QW // 128, HPC], F32, tag="dn")
                        for qt in range(QW // 128):
                            nc.tensor.matmul(
                                dnp[:, qt, :],
                                den2[:, qt * 128:(qt + 1) * 128],
                                ident[:, 0:65:64],
                                start=True, stop=True)
                        nc.vector.reciprocal(
                            denr[:, st0:st0 + QW // 128, :], dnp[:])
                        # dense partial; per-head normalization as
                        # per-partition scalars in the epilogue
                        for stl in range(QW // 128):
                            st = qbase // 128 + stl
                            ssl = slice(st * 128, (st + 1) * 128)
                            for nt in range(HID // 512):
                                nsl = slice(nt * 512, (nt + 1) * 512)
                                psa = pdn.tile([128, 512], F32, tag="dn")
                                nc.tensor.matmul(
                                    psa[:], ctxTz[:, 0, ssl], wd_sb[:, nsl],
                                    start=True, stop=True)
                                psb = pdn.tile([128, 512], F32, tag="dn")
                                nc.tensor.matmul(
                                    psb[:], ctxTz[:, 1, ssl], wd_sb[:, nsl],
                                    start=True, stop=True)
                                ob = osp.tile([128, 512], F32)
                                nc.vector.tensor_scalar_mul(
                                    ob[:], psa[:], denr[:, st, 0:1])
                                ob2 = osp.tile([128, 512], F32, tag="ob2")
                                nc.vector.scalar_tensor_tensor(
                                    ob2[:], psb[:], denr[:, st, 1:2], ob[:],
                                    op0=mybir.AluOpType.mult,
                                    op1=mybir.AluOpType.add)
                                nc.sync.dma_start(
                                    out[ssl, nsl], ob2[:])

    nc.compile()
    return nc


_NC_CACHE = None


def get_nc():
    global _NC_CACHE
    if _NC_CACHE is None:
        _NC_CACHE = build_nc()
    return _NC_CACHE


def make_in_maps(hidden_states, w_qkv, b_qkv, w_dense):
    hs = np.ascontiguousarray(
        np.asarray(hidden_states, dtype=np.float32).reshape(SEQ, HID))
    w_qkv = np.asarray(w_qkv, dtype=np.float32)
    b_qkv = np.asarray(b_qkv, dtype=np.float32)
    w_dense = np.asarray(w_dense, dtype=np.float32)
    # Reference layout: qkv.reshape(B, S, HEADS, 3*D) split on the last
    # axis, i.e. w_qkv columns are per-head [q_h | k_h | v_h] blocks of D.
    wq_cols = np.concatenate(
        [np.arange(h * 3 * D, h * 3 * D + D) for h in range(HEADS)])
    wk_cols = wq_cols + D
    wv_cols = wq_cols + 2 * D
    in_maps = []
    for c in range(NCORES):
        c0 = c * CW
        sel = slice(c0, c0 + CW)
        in_maps.append({
            "hs": hs,
            "wq": np.ascontiguousarray(w_qkv[:, wq_cols[sel]]),
            "wk": np.ascontiguousarray(w_qkv[:, wk_cols[sel]]),
            "wv": np.ascontiguousarray(w_qkv[:, wv_cols[sel]]),
            "bq": np.ascontiguousarray(b_qkv[wq_cols[sel]].reshape(CW, 1)),
            "bk": np.ascontiguousarray(b_qkv[wk_cols[sel]].reshape(CW, 1)),
            "wd": np.ascontiguousarray(w_dense[sel, :]),
        })
    return in_maps


def run(hidden_states, w_qkv, b_qkv, w_dense, b_dense, trace=False):
    nc = get_nc()
    in_maps = make_in_maps(hidden_states, w_qkv, b_qkv, w_dense)
    res = run_bass_kernel_spmd(nc, in_maps, core_ids=list(range(NCORES)),
                               trace=trace)
    acc = res.results[0]["out"].astype(np.float32)
    for c in range(1, NCORES):
        acc = acc + res.results[c]["out"]
    # bias terms that commute to the end: v-bias through dense, dense bias
    b_qkv = np.asarray(b_qkv, dtype=np.float32)
    b_v = np.concatenate(
        [b_qkv[h * 3 * D + 2 * D:h * 3 * D + 3 * D] for h in range(HEADS)])
    acc = acc + (b_v @ np.asarray(w_dense, dtype=np.float32)
                 + np.asarray(b_dense, dtype=np.float32))
    return acc.reshape(B, S, HID).astype(np.float32), res


def kernel(hidden_states, w_qkv, b_qkv, w_dense, b_dense):
    out, _ = run(hidden_states, w_qkv, b_qkv, w_dense, b_dense,
                 trace=bool(os.environ.get("BASS_TRACE")))
    return out


# revision 32
# speedup vs baseline: 1.6673x; 1.0058x over previous
"""Multi-head attention (B=2, S=2048, H=1024, 16 heads) on 8 NeuronCores.

Tensor-parallel sharding: 2 heads per core.  Each core computes QKV for its
heads, full attention over the sequence for its heads, and a partial output
projection (its 128 rows of w_dense).  The host sums the 8 partial outputs
(the all-reduce) and adds the output-side bias terms.

Layout notes (per core), all PE matmuls in plain 128x128 mode (mixing
64-row tiled and 128-row matmuls mode-thrashes the PE and halves its
clock, measured):
  hsT  [hid, seq]       hidden states transposed (PE transpose), streamed
                        in 512-seq windows.
  QTz/KTz [128, h, seq] q/k transposed per head, zero-padded to a full
                        128-partition contraction (rows 64-127 = 0).
  Vn  [128, 32, 2, 66]  v natural: partition = seq within 128-chunk,
                        [chunk, head, dim]; col 64 is 1.0 so the P@V
                        matmul also emits the softmax denominators.
  PT  [128, RING, 1024] exp(scores) ring: partition = k within chunk.
  ctxTz [128, h, seq]   unnormalized context transposed; head 0 in rows
                        0-63, head 1 in rows 64-127, other half zero, so
                        the dense matmul takes full w_dense slices.
  Softmax 1/sums are extracted by a basis-column matmul, reciprocals run
  wide on [128, 8], and the normalization lands in the dense epilogue as
  per-partition, per-head scalars.
"""

import os
import sys
import types

sys.path.insert(0, "/opt/trn_rl_repo")

import numpy as np


def _install_ntff_shim():
    """The trimmed container image lacks ``antenv.axon_hooks``, which
    ``run_bass_kernel_spmd(trace=True)`` needs to capture NTFF profiles
    under axon.  Recreate it from the boot helper + the injected .so."""
    if "antenv.axon_hooks" in sys.modules:
        return
    try:
        from trn_agent_boot.trn_boot import _ntff_profile_via_ctypes
        so = "/opt/axon/libaxon_pjrt.so"
        if not os.path.exists(so):
            return
        hook = _ntff_profile_via_ctypes(so)
        mod = types.ModuleType("antenv.axon_hooks")
        mod.get_axon_ntff_profile_hook = lambda: hook
        mod.set_axon_ntff_profile_hook = lambda h: None
        sys.modules["antenv.axon_hooks"] = mod
    except Exception:
        pass


_install_ntff_shim()

import concourse.bass as bass
import concourse.mybir as mybir
import concourse.tile as tile
from concourse import bacc
from concourse.bass_utils import run_bass_kernel_spmd
from concourse.masks import make_identity

F32 = mybir.dt.float32
F32R = mybir.dt.float32r
EXP = mybir.ActivationFunctionType.Exp

B, S, HID = 2, 2048, 1024
HEADS, D = 16, 64
SEQ = B * S                      # 4096 flattened rows
NCORES = 8
HPC = HEADS // NCORES            # heads per core = 2
CW = HPC * D                     # per-core width = 128
NHB = HID // 128                 # hidden 128-chunks = 8
WSEQ = 512                       # seq window for transpose+QKV
NWIN = SEQ // WSEQ               # 8
QW = 1024                        # q window in attention
NKT = S // 128                   # k chunks per batch = 16
NCH = SEQ // 128                 # global 128-row chunks = 32


def build_nc():
    nc = bacc.Bacc("TRN2", target_bir_lowering=False, debug=False,
                   num_devices=NCORES)

    hs = nc.dram_tensor("hs", [SEQ, HID], F32, kind="ExternalInput")
    wq = nc.dram_tensor("wq", [HID, CW], F32, kind="ExternalInput")
    wk = nc.dram_tensor("wk", [HID, CW], F32, kind="ExternalInput")
    wv = nc.dram_tensor("wv", [HID, CW], F32, kind="ExternalInput")
    bq = nc.dram_tensor("bq", [CW, 1], F32, kind="ExternalInput")
    bk = nc.dram_tensor("bk", [CW, 1], F32, kind="ExternalInput")
    wd = nc.dram_tensor("wd", [CW, HID], F32, kind="ExternalInput")
    out = nc.dram_tensor("out", [SEQ, HID], F32, kind="ExternalOutput")

    RING = 4

    with tile.TileContext(nc) as tc:
        with (
            tc.tile_pool(name="persist", bufs=1) as pp,
            tc.tile_pool(name="pt", bufs=1) as ptp,
        ):
            ident = pp.tile([128, 128], F32)
            make_identity(nc, ident[:])

            wq_sb = pp.tile([128, NHB, CW], F32R)
            wk_sb = pp.tile([128, NHB, CW], F32R)
            wv_sb = pp.tile([128, NHB, CW], F32R)
            for wsb, wdr in ((wq_sb, wq), (wk_sb, wk), (wv_sb, wv)):
                nc.sync.dma_start(
                    wsb[:],
                    wdr.ap().bitcast(F32R).rearrange("(c p) m -> p c m", p=128))
            bq_sb = pp.tile([CW, 1], F32)
            bk_sb = pp.tile([CW, 1], F32)
            nc.sync.dma_start(bq_sb[:], bq[:])
            nc.sync.dma_start(bk_sb[:], bk[:])
            wd_sb = pp.tile([CW, HID], F32R)
            nc.sync.dma_start(wd_sb[:], wd.ap().bitcast(F32R))

            # Everything on the PE stays in plain 128x128 mode.  Per-head
            # operands are zero-padded to a full 128-partition contraction:
            #   QTz/KTz [:, h, :]  rows 0-63 = head h, rows 64-127 = 0
            #   ctxTz   [:, 0, :]  rows 0-63 = head 0 ctx, upper rows 0
            #   ctxTz   [:, 1, :]  rows 64-127 = head 1 ctx, lower rows 0
            # so the dense matmul can take full-width w_dense slices.
            QTz = pp.tile([128, HPC, SEQ], F32R)
            KTz = pp.tile([128, HPC, SEQ], F32R)
            Vn = pp.tile([128, NCH, HPC, 66], F32R)
            ctxTz = pp.tile([128, HPC, SEQ], F32R)
            denr = pp.tile([128, NCH, HPC], F32)   # 1/rowsum, [q%128, st, h]
            den2 = pp.tile([128, QW], F32)  # rows 0/64 = h0/h1 rowsums
            PT = ptp.tile([128, RING, QW], F32R)

            nc.vector.memset(den2[:], 0.0)

            # ones column used by the P@V matmul to emit row sums
            ones_st = pp.tile([128, NCH * HPC], F32)
            nc.vector.memset(ones_st[:], 1.0)
            nc.vector.tensor_copy(
                Vn[:, :, :, 64:65],
                ones_st[:].rearrange("p (c h) -> p c h", c=NCH)
                .rearrange("p c h -> p c h ()"))

            # ---------------- phase 1: transpose hs + QKV projections ----
            with (
                tc.tile_pool(name="hsload", bufs=4) as hlp,
                tc.tile_pool(name="hstw", bufs=2) as hwp,
                tc.tile_pool(name="vtw", bufs=2) as vwp,
                tc.tile_pool(name="zs", bufs=1) as zsp,
                tc.tile_pool(name="ps_tr", bufs=2,
                             space=bass.MemorySpace.PSUM) as ptr,
                tc.tile_pool(name="ps_qkv", bufs=2,
                             space=bass.MemorySpace.PSUM) as pqk,
            ):
                # zero-fill the padded halves (f32r memset is rejected by
                # the ISA checker, so bounce through an f32 staging tile)
                zs = zsp.tile([D, SEQ // 4], F32)
                nc.vector.memset(zs[:], 0.0)
                for z0 in range(0, SEQ, SEQ // 4):
                    zl = slice(z0, z0 + SEQ // 4)
                    for h in range(HPC):
                        nc.gpsimd.tensor_copy(QTz[D:128, h, zl], zs[:])
                        nc.gpsimd.tensor_copy(KTz[D:128, h, zl], zs[:])
                    nc.gpsimd.tensor_copy(ctxTz[D:128, 0, zl], zs[:])
                    nc.gpsimd.tensor_copy(ctxTz[0:D, 1, zl], zs[:])

                hsT = hwp.tile([128, NHB, WSEQ], F32R)
                for w in range(NWIN):
                    r0 = w * WSEQ
                    for sb in range(WSEQ // 128):
                        hsn = hlp.tile([128, HID], F32)
                        nc.sync.dma_start(hsn[:], hs[r0 + sb * 128:
                                                     r0 + (sb + 1) * 128, :])
                        trp = ptr.tile([128, HID], F32, tag="tr")
                        for hb in range(NHB):
                            nc.tensor.transpose(
                                trp[:, hb * 128:(hb + 1) * 128],
                                hsn[:, hb * 128:(hb + 1) * 128],
                                ident[:])
                        if sb % 2 == 0:
                            nc.scalar.copy(
                                hsT[:, :, sb * 128:(sb + 1) * 128],
                                trp[:].rearrange("p (h s) -> p h s", h=NHB))
                        else:
                            nc.vector.tensor_copy(
                                hsT[:, :, sb * 128:(sb + 1) * 128],
                                trp[:].rearrange("p (h s) -> p h s", h=NHB))
                    for tgt in range(3):
                        ps = pqk.tile([128, WSEQ], F32, tag="qkv")
                        wsb = (wq_sb, wk_sb, wv_sb)[tgt]
                        for hb in range(NHB):
                            nc.tensor.matmul(
                                ps[:], wsb[:, hb, :], hsT[:, hb, :],
                                start=(hb == 0), stop=(hb == NHB - 1))
                        if tgt < 2:
                            dst, bias = ((QTz, bq_sb), (KTz, bk_sb))[tgt]
                            for h in range(HPC):
                                nc.vector.tensor_scalar_add(
                                    dst[0:D, h, r0:r0 + WSEQ],
                                    ps[h * D:(h + 1) * D, :],
                                    bias[h * D:(h + 1) * D, 0:1])
                        else:
                            vtw = vwp.tile([128, WSEQ], F32)
                            nc.vector.tensor_copy(vtw[:], ps[:])
                            vps = ptr.tile([128, WSEQ], F32, tag="vtr")
                            for sb2 in range(WSEQ // 128):
                                nc.tensor.transpose(
                                    vps[:, sb2 * 128:(sb2 + 1) * 128],
                                    vtw[:, sb2 * 128:(sb2 + 1) * 128],
                                    ident[:])
                            ch0 = r0 // 128
                            nc.vector.tensor_copy(
                                Vn[:, ch0:ch0 + 4, :, 0:64],
                                vps[:].rearrange("p (c h d) -> p c h d",
                                                 c=4, h=HPC))

            # ---------------- phase 2: attention + output projection -----
            with (
                tc.tile_pool(name="ps_st", bufs=2,
                             space=bass.MemorySpace.PSUM) as pst,
                tc.tile_pool(name="ps_pv", bufs=1,
                             space=bass.MemorySpace.PSUM) as ppv,
                tc.tile_pool(name="ps_dn", bufs=2,
                             space=bass.MemorySpace.PSUM) as pdn,
                tc.tile_pool(name="outst", bufs=4) as osp,
            ):
                for b in range(B):
                    for qw in range(S // QW):
                        qbase = b * S + qw * QW
                        qsl = slice(qbase, qbase + QW)
                        st0 = qbase // 128
                        for hh in range(HPC):
                            pvp = ppv.tile([D + 1, QW], F32, tag="pv")
                            for kt in range(NKT):
                                ch = b * NKT + kt
                                ksl = slice(b * S + kt * 128,
                                            b * S + (kt + 1) * 128)
                                rg = kt % RING
                                stp = pst.tile([128, QW], F32, tag="st")
                                for qh in range(QW // 512):
                                    sl = slice(qh * 512, (qh + 1) * 512)
                                    nc.tensor.matmul(
                                        stp[:, sl], KTz[:, hh, ksl],
                                        QTz[:, hh,
                                            qbase + qh * 512:
                                            qbase + (qh + 1) * 512],
                                        start=True, stop=True)
                                nc.scalar.activation(
                                    PT[:, rg, :], stp[:], EXP, scale=0.125)
                                for qh in range(QW // 512):
                                    sl = slice(qh * 512, (qh + 1) * 512)
                                    nc.tensor.matmul(
                                        pvp[:, sl], Vn[:, ch, hh, 0:65],
                                        PT[:, rg, sl],
                                        start=(kt == 0),
                                        stop=(kt == NKT - 1))
                            # ctx into its head's partition range; keep the
                            # denominator row for the normalization pass
                            nc.vector.tensor_copy(
                                ctxTz[hh * D:(hh + 1) * D, hh, qsl],
                                pvp[0:D, :])
                            if hh == 0:
                                nc.scalar.copy(
                                    den2[0:1, :], pvp[D:D + 1, :])
                            else:
                                nc.vector.tensor_copy(
                                    den2[64:65, :], pvp[D:D + 1, :])
                        # extract both heads' rowsum rows transposed in one
                        # matmul per 128-q block via basis columns 0 and 64
                        dnp = pdn.tile([128, QW // 128, HPC], F32, tag="dn")
                        for qt in range(QW // 128):
                            nc.tensor.matmul(
                                dnp[:, qt, :],
                                den2[:, qt * 128:(qt + 1) * 128],
                                ident[:, 0:65:64],
                                start=True, stop=True)
                        nc.vector.reciprocal(
                            denr[:, st0:st0 + QW // 128, :], dnp[:])
                        # dense partial; per-head normalization as
                        # per-partition scalars in the epilogue
                        for stl in range(QW // 128):
                            st = qbase // 128 + stl
                            ssl = slice(st * 128, (st + 1) * 128)
                            for nt in range(HID // 512):
                                nsl = slice(nt * 512, (nt + 1) * 512)
                                psa = pdn.tile([128, 512], F32, tag="dn")
                                nc.tensor.matmul(
                                    psa[:], ctxTz[:, 0, ssl], wd_sb[:, nsl],
                                    start=True, stop=True)
                                psb = pdn.tile([128, 512], F32, tag="dn")
                                nc.tensor.matmul(
                                    psb[:], ctxTz[:, 1, ssl], wd_sb[:, nsl],
                                    start=True, stop=True)
                                ob = osp.tile([128, 512], F32)
                                nc.vector.tensor_scalar_mul(
                                    ob[:], psa[:], denr[:, st, 0:1])
                                ob2 = osp.tile([128, 512], F32, tag="ob2")
                                nc.vector.scalar_tensor_tensor(
                                    ob2[:], psb[:], denr[:, st, 1:2], ob[:],
                                    op0=mybir.AluOpType.mult,
                                    op1=mybir.AluOpType.add)
                                nc.sync.dma_start(
                                    out[ssl, nsl], ob2[:])

    nc.compile()
    return nc


_NC_CACHE = None


def get_nc():
    global _NC_CACHE
    if _NC_CACHE is None:
        _NC_CACHE = build_nc()
    return _NC_CACHE


def make_in_maps(hidden_states, w_qkv, b_qkv, w_dense):
    hs = np.ascontiguousarray(
        np.asarray(hidden_states, dtype=np.float32).reshape(SEQ, HID))
    w_qkv = np.asarray(w_qkv, dtype=np.float32)
    b_qkv = np.asarray(b_qkv, dtype=np.float32)
    w_dense = np.asarray(w_dense, dtype=np.float32)
    # Reference layout: qkv.reshape(B, S, HEADS, 3*D) split on the last
    # axis, i.e. w_qkv columns are per-head [q_h | k_h | v_h] blocks of D.
    wq_cols = np.concatenate(
        [np.arange(h * 3 * D, h * 3 * D + D) for h in range(HEADS)])
    wk_cols = wq_cols + D
    wv_cols = wq_cols + 2 * D
    in_maps = []
    for c in range(NCORES):
        c0 = c * CW
        sel = slice(c0, c0 + CW)
        in_maps.append({
            "hs": hs,
            "wq": np.ascontiguousarray(w_qkv[:, wq_cols[sel]]),
            "wk": np.ascontiguousarray(w_qkv[:, wk_cols[sel]]),
            "wv": np.ascontiguousarray(w_qkv[:, wv_cols[sel]]),
            "bq": np.ascontiguousarray(b_qkv[wq_cols[sel]].reshape(CW, 1)),
            "bk": np.ascontiguousarray(b_qkv[wk_cols[sel]].reshape(CW, 1)),
            "wd": np.ascontiguousarray(w_dense[sel, :]),
        })
    return in_maps


def run(hidden_states, w_qkv, b_qkv, w_dense, b_dense, trace=False):
    nc = get_nc()
    in_maps = make_in_maps(hidden_states, w_qkv, b_qkv, w_dense)
    res = run_bass_kernel_spmd(nc, in_maps, core_ids=list(range(NCORES)),
                               trace=trace)
    acc = res.results[0]["out"].astype(np.float32)
    for c in range(1, NCORES):
        acc = acc + res.results[c]["out"]
    # bias terms that commute to the end: v-bias through dense, dense bias
    b_qkv = np.asarray(b_qkv, dtype=np.float32)
    b_v = np.concatenate(
        [b_qkv[h * 3 * D + 2 * D:h * 3 * D + 3 * D] for h in range(HEADS)])
    acc = acc + (b_v @ np.asarray(w_dense, dtype=np.float32)
                 + np.asarray(b_dense, dtype=np.float32))
    return acc.reshape(B, S, HID).astype(np.float32), res


def kernel(hidden_states, w_qkv, b_qkv, w_dense, b_dense):
    out, _ = run(hidden_states, w_qkv, b_qkv, w_dense, b_dense,
                 trace=bool(os.environ.get("BASS_TRACE")))
    return out
